# revision 11
# baseline (speedup 1.0000x reference)
"""GAT 2-layer propagation kernel for Trainium2, 8 NeuronCores (SPMD).

Strategy (edge-parallel, dst-node-range sharded across 8 cores):
  - Core c owns dst nodes [c*6250, (c+1)*6250); edges (with self-loops) go to
    the core owning their dst, so each core emits its contiguous output rows
    and no reduction collective is needed.
  - Per layer a DRAM gather table holds one fp16 row per node:
      G1[n] = [h1 x128 | as1 x4 | ad1 x4 | pad]   (512 B rows)
      G2[n] = [h2 x32  | as2    | ad2    | pad]   (256 B rows)
    (h carries the layer bias folded in: softmax weights sum to 1, so adding
    b to every value row adds b to the output.)
  - Per 128-dst tile, edges are packed DENSELY into chunks of 128 slots
    (partition dim), lo-src chunks then hi-src chunks (dma_gather indices
    are int16, so the node table is addressed in two halves).  Dense packing
    needs ~19 chunks/tile vs ~21 for the aligned layout, directly shrinking
    GpSimd descriptor generation (the serial bottleneck, ~8ns/row).  Unused
    tail slots gather row 0; the validity mask zeroes their weight.
  - Each chunk k has a host-precomputed [slot, dst] 0/1 selection mask (and
    its transpose).  maskT @ adt recovers per-slot alpha_dst; mask is the
    stationary operand of the per-chunk segment-sum matmul, which
    accumulates [sum e*h | sum e] in fp32 PSUM across chunks; the softmax
    division happens once per dst at the end.
  - e = exp(leakyrelu(as+ad)) needs no max-subtraction (|alpha| <= ~6 here);
    exp runs in fp32 so stale-slot garbage stays finite, and the validity
    mask zeroes it before the fp16 value multiply.
  - Layer-2 table rows are built inline as each layer-1 output tile
    finishes; an AllGather + relayout replicates the table. The final
    sigmoid runs as one deferred sweep so ACT's activation table stays on
    Exp during edge phases.
"""

import numpy as np

import concourse.bacc as bacc
import concourse.tile as tile
from concourse import mybir
from concourse.bass import IndirectOffsetOnAxis
from concourse.bass_utils import run_bass_kernel_spmd

F32 = mybir.dt.float32
F16 = mybir.dt.float16
I32 = mybir.dt.int32
I16 = mybir.dt.int16
AF = mybir.ActivationFunctionType
OP = mybir.AluOpType

P = 128
HALF = 32768            # int16-addressable rows per dma_gather call


class GATConfig:
    def __init__(self, n, e, in_dim=128, hid=32, heads=4, out_dim=32,
                 neg_slope=0.2, n_cores=8):
        assert in_dim == P and heads * hid == P
        self.N, self.E = n, e
        self.HID = hid
        self.H = heads
        self.OUT = out_dim
        self.NEG = neg_slope
        self.NC = n_cores
        assert n % n_cores == 0
        self.NPC = n // n_cores
        self.NT = (self.NPC + P - 1) // P
        self.LAST = self.NPC - (self.NT - 1) * P
        self.C1 = heads * hid                 # 128
        self.G1W = 256                        # fp16 els/row: h|as|ad|pad
        self.G2W = 128
        self.NNT = (n + P - 1) // P
        self.LASTN = n - (self.NNT - 1) * P


class EdgePlan:
    """Dense chunk structure: per-tile chunk counts (max over cores)."""


def _prep_host(cfg, x, edge_index, W1, a_src1, a_dst1, b1, W2, a_src2,
               a_dst2, b2):
    N, H, HID = cfg.N, cfg.H, cfg.HID
    NPC, NT, NC = cfg.NPC, cfg.NT, cfg.NC

    src = np.concatenate([np.asarray(edge_index[0], dtype=np.int64),
                          np.arange(N, dtype=np.int64)])
    dst = np.concatenate([np.asarray(edge_index[1], dtype=np.int64),
                          np.arange(N, dtype=np.int64)])
    order = np.argsort(dst, kind="stable")
    src, dst = src[order], dst[order]

    core_of = dst // NPC
    tile_of = (dst % NPC) // P
    part_of = (dst % NPC) % P
    is_hi = src >= HALF

    nlo = np.zeros((NC, NT), np.int64)
    nhi = np.zeros((NC, NT), np.int64)
    np.add.at(nlo, (core_of[~is_hi], tile_of[~is_hi]), 1)
    np.add.at(nhi, (core_of[is_hi], tile_of[is_hi]), 1)

    CLs = np.ceil(nlo.max(axis=0) / P).astype(int)     # per-tile, all cores
    CHs = np.ceil(nhi.max(axis=0) / P).astype(int)
    Ss = CLs + CHs
    plan = EdgePlan()
    plan.CLs, plan.CHs = tuple(int(v) for v in CLs), tuple(int(v) for v in CHs)
    plan.Ss = tuple(int(v) for v in Ss)
    plan.SMAX = int(Ss.max())
    olo = np.concatenate([[0], np.cumsum(CLs * 8)])    # idx col offsets
    ohi = np.concatenate([[0], np.cumsum(CHs * 8)])
    ovo = np.concatenate([[0], np.cumsum(Ss)])         # vmask col offsets
    omo = np.concatenate([[0], np.cumsum(Ss * P)])     # mask col offsets
    plan.olo, plan.ohi = tuple(olo.tolist()), tuple(ohi.tolist())
    plan.ovo, plan.omo = tuple(ovo.tolist()), tuple(omo.tolist())
    TLO, THI, TVM, TMK = olo[-1], ohi[-1], ovo[-1], omo[-1]

    def pack16(vals):
        # vals: [n_chunks*P] int16 in slot order j=k*128+p; idx j lives at
        # [j%16, j//16], replicated across the 8 stripes of 16 partitions.
        a = vals.reshape(-1, 16).T.astype(np.int16)   # [16, n/16]
        return np.tile(a, (8, 1))

    per_core = []
    for c in range(NC):
        m = core_of == c
        s_c = src[m].astype(np.int64)
        t_c = tile_of[m]
        p_c = part_of[m]
        ilo = np.zeros((P, TLO), np.int16)
        ihi = np.zeros((P, THI), np.int16)
        vmk = np.zeros((P, TVM), np.float16)
        ovm = np.zeros((P, TMK), np.float16)
        ovmT = np.zeros((P, TMK), np.float16)
        for t in range(NT):
            CLt, CHt = CLs[t], CHs[t]
            mt = t_c == t
            s_t, p_t = s_c[mt], p_c[mt]
            lo_t = s_t < HALF
            for half in (0, 1):
                if half == 0:
                    ss, pp = s_t[lo_t], p_t[lo_t]
                    nch, koff = CLt, 0
                else:
                    ss, pp = s_t[~lo_t] - HALF, p_t[~lo_t]
                    nch, koff = CHt, CLt
                if nch == 0:
                    continue
                iv = np.full(nch * P, -1, np.int16)
                ne = len(ss)
                iv[:ne] = ss
                jj = np.arange(ne)
                kk, sl = jj // P, jj % P
                vmk[sl, ovo[t] + koff + kk] = 1.0
                ovm[sl, omo[t] + (koff + kk) * P + pp] = 1.0
                ovmT[pp, omo[t] + (koff + kk) * P + sl] = 1.0
                pk = pack16(iv)
                if half == 0:
                    ilo[:, olo[t]:olo[t] + nch * 8] = pk
                else:
                    ihi[:, ohi[t]:ohi[t] + nch * 8] = pk

        ntrim = np.zeros((1, 2 * NT), np.int32)
        ntrim[0, 0::2] = np.maximum(nlo[c], 1)
        ntrim[0, 1::2] = np.maximum(nhi[c], 1)
        adrows = (c * NPC + np.arange(NT)[None, :] * P
                  + np.arange(P)[:, None]).astype(np.int32)
        np.clip(adrows, 0, N + P - 1, out=adrows)
        per_core.append({
            "idxlo": np.ascontiguousarray(ilo),
            "idxhi": np.ascontiguousarray(ihi),
            "vmask": np.ascontiguousarray(vmk),
            "ovmask": np.ascontiguousarray(ovm),
            "ovmaskT": np.ascontiguousarray(ovmT),
            "adrows": np.ascontiguousarray(adrows),
            "ntrim": np.ascontiguousarray(ntrim),
        })

    # block-diagonal attention projectors: as1 = h1 @ asrc_blk
    asrc_blk = np.zeros((cfg.C1, H), np.float32)
    adst_blk = np.zeros((cfg.C1, H), np.float32)
    for h in range(H):
        asrc_blk[h * HID:(h + 1) * HID, h] = a_src1[h]
        adst_blk[h * HID:(h + 1) * HID, h] = a_dst1[h]

    b1row = np.zeros((1, cfg.C1 + 2 * H), np.float32)
    b1row[0, :cfg.C1] = b1
    b2row = np.zeros((1, cfg.OUT + 2), np.float32)
    b2row[0, :cfg.OUT] = b2

    common = {
        "xT": np.ascontiguousarray(np.asarray(x, np.float16).T),
        "W1h": np.ascontiguousarray(np.asarray(W1, np.float16)),
        "W1T": np.ascontiguousarray(np.asarray(W1, np.float32).T),
        "asrcblk": asrc_blk, "adstblk": adst_blk, "b1row": b1row,
        "W2h": np.ascontiguousarray(np.asarray(W2, np.float16)),
        "W2T": np.ascontiguousarray(np.asarray(W2, np.float32).T),
        "a2src": np.ascontiguousarray(
            np.asarray(a_src2, np.float32).reshape(-1, 1)),
        "a2dst": np.ascontiguousarray(
            np.asarray(a_dst2, np.float32).reshape(-1, 1)),
        "b2row": b2row,
        "identh": np.eye(P, dtype=np.float16),
        "onesrow": np.ones((1, P), np.float32),
    }
    return plan, common, per_core


def _build(cfg, plan):
    N, H, HID, C1 = cfg.N, cfg.H, cfg.HID, cfg.C1
    NT, NPC, NNT = cfg.NT, cfg.NPC, cfg.NNT
    C2 = cfg.OUT
    CLs, CHs, Ss, SMAX = plan.CLs, plan.CHs, plan.Ss, plan.SMAX
    olo, ohi, ovo, omo = plan.olo, plan.ohi, plan.ovo, plan.omo
    G1W, G2W = cfg.G1W, cfg.G2W
    G1C = C1 + 2 * H                       # 136 used cols in G1 rows
    G2C = C2 + 2                           # 34 used cols in G2 rows

    nc = bacc.Bacc("TRN2", target_bir_lowering=False, debug=False,
                   num_devices=cfg.NC)

    def din(name, shape, dt=F32):
        return nc.dram_tensor(name, shape, dt, kind="ExternalInput").ap()

    xT = din("xT", [P, N], F16)
    W1h = din("W1h", [P, C1], F16)
    W1T = din("W1T", [C1, P])
    asrcblk = din("asrcblk", [C1, H])
    adstblk = din("adstblk", [C1, H])
    b1row = din("b1row", [1, G1C])
    W2h = din("W2h", [C1, C2], F16)
    W2T = din("W2T", [C2, C1])
    a2src = din("a2src", [C2, 1])
    a2dst = din("a2dst", [C2, 1])
    b2row = din("b2row", [1, G2C])
    identh = din("identh", [P, P], F16)
    onesrow = din("onesrow", [1, P])
    idxlo = din("idxlo", [P, olo[NT]], I16)
    idxhi = din("idxhi", [P, ohi[NT]], I16)
    vmask = din("vmask", [P, ovo[NT]], F16)
    ovmask = din("ovmask", [P, omo[NT]], F16)
    ovmaskT = din("ovmaskT", [P, omo[NT]], F16)
    adrows = din("adrows", [P, NT], I32)
    ntrim = din("ntrim", [1, 2 * NT], I32)

    out = nc.dram_tensor("out", [NT * P, C2], F32, kind="ExternalOutput").ap()

    G1 = nc.dram_tensor("G1", [N + P, G1W], F16).ap()
    G2 = nc.dram_tensor("G2", [N + P, G2W], F16).ap()
    G2c = nc.dram_tensor("G2c", [NPC, G2C], F16).ap()
    TSPLIT = (24, 42)                      # allgather after these tiles
    ROW0 = (0, TSPLIT[0] * P, TSPLIT[1] * P, NPC)
    G2cfs = [nc.dram_tensor(f"G2cf{j}",
                            [cfg.NC * (ROW0[j + 1] - ROW0[j]), G2C], F16,
                            addr_space="Shared").ap() for j in range(3)]
    PRE = nc.dram_tensor("PRE", [NT * P, C2], F32).ap()

    with tile.TileContext(nc) as tc:
        with tc.tile_pool(name="const", bufs=1) as const:
            # ---- constants / fused weight tables --------------------------
            with tc.tile_pool(name="cpsum", bufs=1, space="PSUM") as cpsum:
                w1ext = const.tile([P, G1C], F16)   # [W1 | W1@Asrc | W1@Adst]
                nc.sync.dma_start(out=w1ext[:, 0:C1], in_=W1h)
                w1t = const.tile([P, P], F32)
                nc.sync.dma_start(out=w1t[:], in_=W1T)
                ablk = const.tile([P, 2 * H], F32)
                nc.sync.dma_start(out=ablk[:, 0:H], in_=asrcblk)
                nc.sync.dma_start(out=ablk[:, H:2 * H], in_=adstblk)
                pw = cpsum.tile([P, 2 * H], F32, space="PSUM")
                nc.tensor.matmul(pw[:], lhsT=w1t[:], rhs=ablk[:], start=True,
                                 stop=True)
                nc.vector.tensor_copy(out=w1ext[:, C1:C1 + 2 * H], in_=pw[:])

                w2ext = const.tile([P, G2C], F16)   # [W2 | W2@a2s | W2@a2d]
                nc.sync.dma_start(out=w2ext[:, 0:C2], in_=W2h)
                w2t = const.tile([C2, C1], F32)
                nc.sync.dma_start(out=w2t[:], in_=W2T)
                a2 = const.tile([C2, 2], F32)
                nc.sync.dma_start(out=a2[:, 0:1], in_=a2src)
                nc.sync.dma_start(out=a2[:, 1:2], in_=a2dst)
                pw2 = cpsum.tile([P, 2], F32, space="PSUM")
                nc.tensor.matmul(pw2[:], lhsT=w2t[:], rhs=a2[:], start=True,
                                 stop=True)
                nc.vector.tensor_copy(out=w2ext[:, C2:C2 + 2], in_=pw2[:])

                # broadcast bias rows to all 128 partitions (ones @ brow)
                b1sb = const.tile([1, G1C], F32)
                nc.sync.dma_start(out=b1sb[:], in_=b1row)
                b2sb = const.tile([1, G2C], F32)
                nc.sync.dma_start(out=b2sb[:], in_=b2row)
                onesb = const.tile([1, P], F32)
                nc.sync.dma_start(out=onesb[:], in_=onesrow)
                b1rep = const.tile([P, G1C], F32)
                pb1 = cpsum.tile([P, G1C], F32, space="PSUM")
                nc.tensor.matmul(pb1[:], lhsT=onesb[:], rhs=b1sb[:],
                                 start=True, stop=True)
                nc.vector.tensor_copy(out=b1rep[:], in_=pb1[:])
                b2rep = const.tile([P, G2C], F32)
                pb2 = cpsum.tile([P, G2C], F32, space="PSUM")
                nc.tensor.matmul(pb2[:], lhsT=onesb[:], rhs=b2sb[:],
                                 start=True, stop=True)
                nc.vector.tensor_copy(out=b2rep[:], in_=pb2[:])

                idsb = const.tile([P, P], F16)
                nc.sync.dma_start(out=idsb[:], in_=identh)
                adr = const.tile([P, NT], I32)
                nc.sync.dma_start(out=adr[:], in_=adrows)
                # prefetch all per-tile gather indices / validity masks
                ilosb = const.tile([P, olo[NT]], I16)
                nc.sync.dma_start(out=ilosb[:], in_=idxlo)
                ihisb = const.tile([P, ohi[NT]], I16)
                nc.sync.dma_start(out=ihisb[:], in_=idxhi)
                vmsb = const.tile([P, ovo[NT]], F16)
                nc.sync.dma_start(out=vmsb[:], in_=vmask)
                adt2sb = const.tile([P, NT], F16)
                nc.vector.memset(adt2sb[:], 0.0)
                ntrimsb = const.tile([1, 2 * NT], I32)
                nc.sync.dma_start(out=ntrimsb[:], in_=ntrim)
            rglo = nc.gpsimd.alloc_register("ntrim_lo")
            rghi = nc.gpsimd.alloc_register("ntrim_hi")

            # ---- phase A: G1 rows -----------------------------------------
            with (
                tc.tile_pool(name="pa", bufs=3) as pa,
                tc.tile_pool(name="pap", bufs=4, space="PSUM") as pap,
            ):
                zz = pa.tile([P, G1W], F16, tag="zz")
                nc.vector.memset(zz[:], 0.0)
                nc.sync.dma_start(out=G1[N:N + P, :], in_=zz[:])
                nc.sync.dma_start(out=G2[N:N + P, :], in_=zz[:, 0:G2W])
                GA = 8
                for i0 in range(0, NNT, GA):
                    gg = min(GA, NNT - i0)
                    wid = (gg - 1) * P + (P if i0 + gg < NNT else cfg.LASTN)
                    xt = pa.tile([P, GA * P], F16, tag="xt")
                    nc.sync.dma_start(out=xt[:, 0:wid],
                                      in_=xT[:, i0 * P:i0 * P + wid])
                    g1h = pa.tile([P, GA * G1C], F16, tag="g1h")
                    for g in range(gg):
                        nn = P if i0 + g < NNT - 1 else cfg.LASTN
                        ps = pap.tile([P, G1C], F32, space="PSUM", tag="ps")
                        nc.tensor.matmul(ps[:nn, :],
                                         lhsT=xt[:, g * P:g * P + nn],
                                         rhs=w1ext[:], start=True, stop=True)
                        nc.vector.tensor_tensor(
                            out=g1h[:nn, g * G1C:(g + 1) * G1C],
                            in0=ps[:nn, :], in1=b1rep[:nn, :], op=OP.add)
                    if gg == GA and wid == GA * P:
                        nc.scalar.dma_start(
                            out=G1[i0 * P:(i0 + GA) * P, 0:G1C]
                                .rearrange("(g p) c -> p g c", g=GA),
                            in_=g1h[:].rearrange("p (g c) -> p g c", g=GA))
                    else:
                        for g in range(gg):
                            nn = P if i0 + g < NNT - 1 else cfg.LASTN
                            nc.scalar.dma_start(
                                out=G1[(i0 + g) * P:(i0 + g) * P + nn, 0:G1C],
                                in_=g1h[:nn, g * G1C:(g + 1) * G1C])

            # ---- phase B: layer-1 edges + layer-2 row build ---------------
            with (
                tc.tile_pool(name="pbig", bufs=3) as pbig,
                tc.tile_pool(name="pxx", bufs=2) as pxx,
                tc.tile_pool(name="pmed", bufs=2) as pmed,
                tc.tile_pool(name="pmsk", bufs=2) as pmsk,
                tc.tile_pool(name="pbp", bufs=2, space="PSUM") as pbp,
                tc.tile_pool(name="pbpa", bufs=2, space="PSUM") as pbpa,
                tc.tile_pool(name="pcp", bufs=1, space="PSUM") as pcp,
                tc.tile_pool(name="pcpt", bufs=1, space="PSUM") as pcpt,
            ):
                for t in range(NT):
                    ndst = P if t < NT - 1 else cfg.LAST
                    CLt, CHt, St = CLs[t], CHs[t], Ss[t]
                    vg = pbig.tile([P, SMAX * G1W], F16, tag="vg")
                    vg3 = vg[:].rearrange("p (k c) -> p k c", c=G1W)
                    if t < 3:
                        nc.vector.memset(vg[:], 0.0)
                    if CLt:
                        nc.gpsimd.load(rglo, ntrimsb[0:1, 2 * t:2 * t + 1])
                        nc.gpsimd.dma_gather(
                            out_ap=vg3[:, 0:CLt, :],
                            in_ap=G1[0:min(HALF, N + P), :],
                            idxs_ap=ilosb[:, olo[t]:olo[t] + CLt * 8],
                            num_idxs=CLt * P, num_idxs_reg=rglo,
                            elem_size=G1W, single_packet=False)
                    if CHt:
                        nc.gpsimd.load(rghi, ntrimsb[0:1, 2 * t + 1:2 * t + 2])
                        nc.gpsimd.dma_gather(
                            out_ap=vg3[:, CLt:St, :], in_ap=G1[HALF:N + P, :],
                            idxs_ap=ihisb[:, ohi[t]:ohi[t] + CHt * 8],
                            num_idxs=CHt * P, num_idxs_reg=rghi,
                            elem_size=G1W, single_packet=False)
                    # alpha_dst values of this tile's dsts, per partition
                    adt = pmed.tile([P, H], F16, tag="adt")
                    nc.gpsimd.indirect_dma_start(
                        out=adt[:], out_offset=None, in_=G1,
                        in_offset=IndirectOffsetOnAxis(ap=adr[:, t:t + 1],
                                                       axis=0),
                        element_offset=C1 + H)
                    msk = pmsk.tile([P, SMAX * P], F16, tag="msk")
                    nc.sync.dma_start(
                        out=msk[:, 0:St * P],
                        in_=ovmask[:, omo[t]:omo[t] + St * P])
                    mskT = pmsk.tile([P, SMAX * P], F16, tag="mskT")
                    nc.scalar.dma_start(
                        out=mskT[:, 0:St * P],
                        in_=ovmaskT[:, omo[t]:omo[t] + St * P])

                    # per-slot alpha_dst for every chunk: maskT @ adt
                    adp = pbpa.tile([P, SMAX * H], F32, space="PSUM",
                                    tag="adp")
                    for k in range(St):
                        nc.tensor.matmul(
                            adp[:, k * H:(k + 1) * H],
                            lhsT=mskT[:, k * P:(k + 1) * P],
                            rhs=adt[:], start=True, stop=True)
                    alp = pmed.tile([P, SMAX * H], F16, tag="alp")
                    alp3 = alp[:].rearrange("p (k h) -> p k h", h=H)
                    nc.vector.tensor_tensor(
                        out=alp3[:, 0:St, :],
                        in0=vg3[:, 0:St, C1:C1 + H],
                        in1=adp[:].rearrange("p (k h) -> p k h", h=H)
                            [:, 0:St, :],
                        op=OP.add)
                    # e = exp(lrelu(alpha)) * vmask
                    asc = pmed.tile([P, SMAX * H], F16, tag="asc")
                    nc.vector.tensor_scalar(out=asc[:, 0:St * H],
                                            in0=alp[:, 0:St * H],
                                            scalar1=cfg.NEG, scalar2=None,
                                            op0=OP.mult)
                    lrl = pmed.tile([P, SMAX * H], F16, tag="lrl")
                    nc.vector.tensor_tensor(out=lrl[:, 0:St * H],
                                            in0=alp[:, 0:St * H],
                                            in1=asc[:, 0:St * H], op=OP.max)
                    ee = pmed.tile([P, SMAX * H], F32, tag="ee")
                    nc.scalar.activation(out=ee[:, 0:St * H],
                                         in_=lrl[:, 0:St * H], func=AF.Exp)
                    eeh = pmed.tile([P, SMAX * H], F16, tag="eeh")
                    nc.vector.tensor_tensor(
                        out=eeh[:, 0:St * H].rearrange("p (k h) -> p k h",
                                                       h=H),
                        in0=ee[:, 0:St * H].rearrange("p (k h) -> p k h",
                                                      h=H),
                        in1=vmsb[:, ovo[t]:ovo[t] + St]
                            .rearrange("p (k o) -> p k o", o=1)
                            .to_broadcast([P, St, H]),
                        op=OP.mult)
                    eeh3 = eeh[:].rearrange("p (k h) -> p k h", h=H)
                    # rhs = [e*h | e]
                    xx = pxx.tile([P, SMAX * (C1 + H)], F16, tag="xx")
                    xx3 = xx[:].rearrange("p (k c) -> p k c", c=C1 + H)
                    nc.vector.tensor_copy(out=xx3[:, 0:St, C1:C1 + H],
                                          in_=eeh3[:, 0:St, :])
                    nc.vector.tensor_tensor(
                        out=xx3[:, 0:St, 0:C1].rearrange(
                            "p k (h c) -> p k h c", c=HID),
                        in0=vg3[:, 0:St, 0:C1].rearrange(
                            "p k (h c) -> p k h c", c=HID),
                        in1=eeh[:, 0:St * H].rearrange(
                            "p (k h o) -> p k h o", h=H, o=1)
                            .to_broadcast([P, St, H, HID]),
                        op=OP.mult)
                    ps = pbp.tile([P, C1 + H], F32, space="PSUM", tag="ps")
                    for k in range(St):
                        nc.tensor.matmul(ps[:], lhsT=msk[:, k * P:(k + 1) * P],
                                         rhs=xx3[:, k, :],
                                         start=(k == 0), stop=(k == St - 1))
                    rec = pmed.tile([P, H], F32, tag="rec")
                    nc.vector.reciprocal(out=rec[:ndst, :],
                                         in_=ps[:ndst, C1:C1 + H])
                    o1 = pmed.tile([P, C1], F16, tag="o1")
                    if ndst < P:
                        nc.vector.memset(o1[:], 0.0)
                    for h in range(H):
                        nc.vector.tensor_scalar(
                            out=o1[:ndst, h * HID:(h + 1) * HID],
                            in0=ps[:ndst, h * HID:(h + 1) * HID],
                            scalar1=rec[:ndst, h:h + 1], scalar2=0.0,
                            op0=OP.mult, op1=OP.max)
                    # layer-2 row build: transpose + project
                    tp = pcpt.tile([P, P], F16, space="PSUM", tag="tp")
                    nc.tensor.transpose(out=tp[:], in_=o1[:],
                                        identity=idsb[:])
                    o1t = pmed.tile([P, P], F16, tag="o1t")
                    nc.vector.tensor_copy(out=o1t[:], in_=tp[:])
                    hp = pcp.tile([P, G2C], F32, space="PSUM", tag="hp")
                    nc.tensor.matmul(hp[:], lhsT=o1t[:], rhs=w2ext[:],
                                     start=True, stop=True)
                    g2h = pmed.tile([P, G2C], F16, tag="g2h")
                    nc.vector.tensor_tensor(out=g2h[:ndst, :],
                                            in0=hp[:ndst, :],
                                            in1=b2rep[:ndst, :], op=OP.add)
                    nc.vector.tensor_copy(out=adt2sb[:ndst, t:t + 1],
                                          in_=g2h[:ndst, C2 + 1:C2 + 2])
                    nc.sync.dma_start(out=G2c[t * P:t * P + ndst, :],
                                      in_=g2h[:ndst, :])
                    if t + 1 in TSPLIT:
                        # partial table replication overlaps the rest of B
                        j = TSPLIT.index(t + 1)
                        r0, r1 = ROW0[j], ROW0[j + 1]
                        w = r1 - r0
                        nc.gpsimd.collective_compute(
                            "AllGather", OP.bypass,
                            replica_groups=[list(range(cfg.NC))],
                            ins=[G2c[r0:r1, :]], outs=[G2cfs[j]])
                        for cc in range(cfg.NC):
                            nc.sync.dma_start(
                                out=G2[cc * NPC + r0:cc * NPC + r1, 0:G2C],
                                in_=G2cfs[j][cc * w:(cc + 1) * w, :])

            r0, r1 = ROW0[2], ROW0[3]
            w = r1 - r0
            nc.gpsimd.collective_compute(
                "AllGather", OP.bypass,
                replica_groups=[list(range(cfg.NC))],
                ins=[G2c[r0:r1, :]], outs=[G2cfs[2]])
            for cc in range(cfg.NC):
                nc.sync.dma_start(
                    out=G2[cc * NPC + r0:cc * NPC + r1, 0:G2C],
                    in_=G2cfs[2][cc * w:(cc + 1) * w, :])

            # ---- phase D: layer-2 edge aggregation (1 head) ---------------
            with (
                tc.tile_pool(name="pdig", bufs=3) as pdig,
                tc.tile_pool(name="pdxx", bufs=2) as pdxx,
                tc.tile_pool(name="pdmd", bufs=2) as pdmd,
                tc.tile_pool(name="pdmk", bufs=2) as pdmk,
                tc.tile_pool(name="pdp", bufs=2, space="PSUM") as pdp,
                tc.tile_pool(name="pdpa", bufs=2, space="PSUM") as pdpa,
            ):
                for t in range(NT):
                    ndst = P if t < NT - 1 else cfg.LAST
                    CLt, CHt, St = CLs[t], CHs[t], Ss[t]
                    vg = pdig.tile([P, SMAX * G2W], F16, tag="vg2")
                    vg3 = vg[:].rearrange("p (k c) -> p k c", c=G2W)
                    if t < 3:
                        nc.vector.memset(vg[:], 0.0)
                    if CLt:
                        nc.gpsimd.load(rglo, ntrimsb[0:1, 2 * t:2 * t + 1])
                        nc.gpsimd.dma_gather(
                            out_ap=vg3[:, 0:CLt, :],
                            in_ap=G2[0:min(HALF, N + P), :],
                            idxs_ap=ilosb[:, olo[t]:olo[t] + CLt * 8],
                            num_idxs=CLt * P, num_idxs_reg=rglo,
                            elem_size=G2W, single_packet=False)
                    if CHt:
                        nc.gpsimd.load(rghi, ntrimsb[0:1, 2 * t + 1:2 * t + 2])
                        nc.gpsimd.dma_gather(
                            out_ap=vg3[:, CLt:St, :], in_ap=G2[HALF:N + P, :],
                            idxs_ap=ihisb[:, ohi[t]:ohi[t] + CHt * 8],
                            num_idxs=CHt * P, num_idxs_reg=rghi,
                            elem_size=G2W, single_packet=False)
                    adt = adt2sb[:, t:t + 1]
                    msk = pdmk.tile([P, SMAX * P], F16, tag="msk2")
                    nc.sync.dma_start(
                        out=msk[:, 0:St * P],
                        in_=ovmask[:, omo[t]:omo[t] + St * P])
                    mskT = pdmk.tile([P, SMAX * P], F16, tag="mskT2")
                    nc.scalar.dma_start(
                        out=mskT[:, 0:St * P],
                        in_=ovmaskT[:, omo[t]:omo[t] + St * P])

                    adp = pdpa.tile([P, SMAX], F32, space="PSUM", tag="adp2")
                    for k in range(St):
                        nc.tensor.matmul(
                            adp[:, k:k + 1],
                            lhsT=mskT[:, k * P:(k + 1) * P],
                            rhs=adt, start=True, stop=True)
                    alp = pdmd.tile([P, SMAX], F16, tag="alp2")
                    alp3 = alp[:].rearrange("p (k o) -> p k o", o=1)
                    nc.vector.tensor_tensor(
                        out=alp3[:, 0:St, :],
                        in0=vg3[:, 0:St, C2:C2 + 1],
                        in1=adp[:].rearrange("p (k o) -> p k o", o=1)
                            [:, 0:St, :],
                        op=OP.add)
                    asc = pdmd.tile([P, SMAX], F16, tag="asc2")
                    nc.vector.tensor_scalar(out=asc[:, 0:St],
                                            in0=alp[:, 0:St],
                                            scalar1=cfg.NEG, scalar2=None,
                                            op0=OP.mult)
                    lrl = pdmd.tile([P, SMAX], F16, tag="lrl2")
                    nc.vector.tensor_tensor(out=lrl[:, 0:St],
                                            in0=alp[:, 0:St],
                                            in1=asc[:, 0:St], op=OP.max)
                    ee = pdmd.tile([P, SMAX], F32, tag="ee2")
                    nc.scalar.activation(out=ee[:, 0:St], in_=lrl[:, 0:St],
                                         func=AF.Exp)
                    eeh = pdmd.tile([P, SMAX], F16, tag="eeh2")
                    nc.vector.tensor_tensor(out=eeh[:, 0:St],
                                            in0=ee[:, 0:St],
                                            in1=vmsb[:, ovo[t]:ovo[t] + St],
                                            op=OP.mult)
                    xx = pdxx.tile([P, SMAX * (C2 + 1)], F16, tag="xx2")
                    xx3 = xx[:].rearrange("p (k c) -> p k c", c=C2 + 1)
                    nc.vector.tensor_copy(
                        out=xx3[:, 0:St, C2:C2 + 1],
                        in_=eeh[:, 0:St].rearrange("p (k o) -> p k o", o=1))
                    nc.vector.tensor_tensor(
                        out=xx3[:, 0:St, 0:C2],
                        in0=vg3[:, 0:St, 0:C2],
                        in1=eeh[:, 0:St].rearrange("p (k o) -> p k o", o=1)
                            .to_broadcast([P, St, C2]),
                        op=OP.mult)
                    ps = pdp.tile([P, C2 + 1], F32, space="PSUM", tag="ps2")
                    for k in range(St):
                        nc.tensor.matmul(ps[:], lhsT=msk[:, k * P:(k + 1) * P],
                                         rhs=xx3[:, k, :],
                                         start=(k == 0), stop=(k == St - 1))
                    rec = pdmd.tile([P, 1], F32, tag="rec2")
                    nc.vector.reciprocal(out=rec[:ndst, :],
                                         in_=ps[:ndst, C2:C2 + 1])
                    o2 = pdmd.tile([P, C2], F32, tag="o2")
                    nc.vector.tensor_scalar(out=o2[:ndst, :],
                                            in0=ps[:ndst, 0:C2],
                                            scalar1=rec[:ndst, :],
                                            scalar2=None, op0=OP.mult)
                    nc.sync.dma_start(out=PRE[t * P:t * P + ndst, :],
                                      in_=o2[:ndst, :])

            # ---- phase E: one sigmoid sweep -------------------------------
            with tc.tile_pool(name="pe", bufs=2) as pe:
                FW = NT * C2
                pre_f = PRE.rearrange("(a b) c -> a (b c)", a=P)
                out_f = out.rearrange("(a b) c -> a (b c)", a=P)
                pei = pe.tile([P, FW], F32, tag="pei")
                nc.sync.dma_start(out=pei[:], in_=pre_f)
                peo = pe.tile([P, FW], F32, tag="peo")
                nc.scalar.activation(out=peo[:], in_=pei[:], func=AF.Sigmoid)
                nc.sync.dma_start(out=out_f, in_=peo[:])

    nc.compile()
    return nc


_CACHE: dict = {}


def _get_module(cfg, plan):
    key = (cfg.N, cfg.E, plan.CLs, plan.CHs)
    if key not in _CACHE:
        _CACHE[key] = _build(cfg, plan)
    return _CACHE[key]


def _run(cfg, inputs, trace=False):
    plan, common, per_core = _prep_host(
        cfg, inputs["x"], inputs["edge_index"], inputs["W1"],
        inputs["a_src1"], inputs["a_dst1"], inputs["b1"], inputs["W2"],
        inputs["a_src2"], inputs["a_dst2"], inputs["b2"])
    nc = _get_module(cfg, plan)
    in_maps = [dict(common, **pc) for pc in per_core]
    res = run_bass_kernel_spmd(nc, in_maps, core_ids=list(range(cfg.NC)),
                               trace=trace)
    shards = [np.asarray(res.results[c]["out"])[:cfg.NPC]
              for c in range(cfg.NC)]
    full = np.concatenate(shards, axis=0).astype(np.float32)
    return (full, res) if trace else full


def kernel(**inputs) -> np.ndarray:
    cfg = GATConfig(n=50000, e=800000)
    return _run(cfg, inputs)


# revision 13
# speedup vs baseline: 1.0287x; 1.0287x over previous
"""GAT 2-layer propagation kernel for Trainium2, 8 NeuronCores (SPMD).

Strategy (edge-parallel, dst-node-range sharded across 8 cores):
  - Core c owns dst nodes [c*6250, (c+1)*6250); edges (with self-loops) go to
    the core owning their dst, so each core emits its contiguous output rows
    and no reduction collective is needed.
  - Per layer a DRAM gather table holds one fp16 row per node:
      G1[n] = [h1 x128 | as1 x4 | ad1 x4 | pad]   (512 B rows)
      G2[n] = [h2 x32  | as2    | ad2    | pad]   (256 B rows)
    (h carries the layer bias folded in: softmax weights sum to 1, so adding
    b to every value row adds b to the output.)
  - Per 128-dst tile, edges are packed DENSELY into chunks of 128 slots
    (partition dim), lo-src chunks then hi-src chunks (dma_gather indices
    are int16, so the node table is addressed in two halves).  Dense packing
    needs ~19 chunks/tile vs ~21 for the aligned layout, directly shrinking
    GpSimd descriptor generation (the serial bottleneck, ~8ns/row).  Unused
    tail slots gather row 0; the validity mask zeroes their weight.
  - Each chunk k has a host-precomputed [slot, dst] 0/1 selection mask (and
    its transpose).  maskT @ adt recovers per-slot alpha_dst; mask is the
    stationary operand of the per-chunk segment-sum matmul, which
    accumulates [sum e*h | sum e] in fp32 PSUM across chunks; the softmax
    division happens once per dst at the end.
  - e = exp(leakyrelu(as+ad)) needs no max-subtraction (|alpha| <= ~6 here);
    exp runs in fp32 so stale-slot garbage stays finite, and the validity
    mask zeroes it before the fp16 value multiply.
  - Layer-2 table rows are built inline as each layer-1 output tile
    finishes; an AllGather + relayout replicates the table. The final
    sigmoid runs as one deferred sweep so ACT's activation table stays on
    Exp during edge phases.
"""

import numpy as np

import concourse.bacc as bacc
import concourse.tile as tile
from concourse import mybir
from concourse.bass import IndirectOffsetOnAxis
from concourse.bass_utils import run_bass_kernel_spmd

F32 = mybir.dt.float32
F16 = mybir.dt.float16
I32 = mybir.dt.int32
I16 = mybir.dt.int16
AF = mybir.ActivationFunctionType
OP = mybir.AluOpType

P = 128
HALF = 32768            # int16-addressable rows per dma_gather call


class GATConfig:
    def __init__(self, n, e, in_dim=128, hid=32, heads=4, out_dim=32,
                 neg_slope=0.2, n_cores=8):
        assert in_dim == P and heads * hid == P
        self.N, self.E = n, e
        self.HID = hid
        self.H = heads
        self.OUT = out_dim
        self.NEG = neg_slope
        self.NC = n_cores
        assert n % n_cores == 0
        self.NPC = n // n_cores
        self.NT = (self.NPC + P - 1) // P
        self.LAST = self.NPC - (self.NT - 1) * P
        self.C1 = heads * hid                 # 128
        self.G1W = 256                        # fp16 els/row: h|as|ad|pad
        self.G2W = 128
        self.NNT = (n + P - 1) // P
        self.LASTN = n - (self.NNT - 1) * P


class EdgePlan:
    """Dense chunk structure: per-tile chunk counts (max over cores)."""


def _prep_host(cfg, x, edge_index, W1, a_src1, a_dst1, b1, W2, a_src2,
               a_dst2, b2):
    N, H, HID = cfg.N, cfg.H, cfg.HID
    NPC, NT, NC = cfg.NPC, cfg.NT, cfg.NC

    src = np.concatenate([np.asarray(edge_index[0], dtype=np.int64),
                          np.arange(N, dtype=np.int64)])
    dst = np.concatenate([np.asarray(edge_index[1], dtype=np.int64),
                          np.arange(N, dtype=np.int64)])
    order = np.argsort(dst, kind="stable")
    src, dst = src[order], dst[order]

    core_of = dst // NPC
    tile_of = (dst % NPC) // P
    part_of = (dst % NPC) % P
    is_hi = src >= HALF

    nlo = np.zeros((NC, NT), np.int64)
    nhi = np.zeros((NC, NT), np.int64)
    np.add.at(nlo, (core_of[~is_hi], tile_of[~is_hi]), 1)
    np.add.at(nhi, (core_of[is_hi], tile_of[is_hi]), 1)

    CLs = np.ceil(nlo.max(axis=0) / P).astype(int)     # per-tile, all cores
    CHs = np.ceil(nhi.max(axis=0) / P).astype(int)
    Ss = CLs + CHs
    plan = EdgePlan()
    plan.CLs, plan.CHs = tuple(int(v) for v in CLs), tuple(int(v) for v in CHs)
    plan.Ss = tuple(int(v) for v in Ss)
    plan.SMAX = int(Ss.max())
    olo = np.concatenate([[0], np.cumsum(CLs * 8)])    # idx col offsets
    ohi = np.concatenate([[0], np.cumsum(CHs * 8)])
    ovo = np.concatenate([[0], np.cumsum(Ss)])         # vmask col offsets
    omo = np.concatenate([[0], np.cumsum(Ss * P)])     # mask col offsets
    plan.olo, plan.ohi = tuple(olo.tolist()), tuple(ohi.tolist())
    plan.ovo, plan.omo = tuple(ovo.tolist()), tuple(omo.tolist())
    TLO, THI, TVM, TMK = olo[-1], ohi[-1], ovo[-1], omo[-1]

    def pack16(vals):
        # vals: [n_chunks*P] int16 in slot order j=k*128+p; idx j lives at
        # [j%16, j//16], replicated across the 8 stripes of 16 partitions.
        a = vals.reshape(-1, 16).T.astype(np.int16)   # [16, n/16]
        return np.tile(a, (8, 1))

    per_core = []
    for c in range(NC):
        m = core_of == c
        s_c = src[m].astype(np.int64)
        t_c = tile_of[m]
        p_c = part_of[m]
        ilo = np.zeros((P, TLO), np.int16)
        ihi = np.zeros((P, THI), np.int16)
        vmk = np.zeros((P, TVM), np.float16)
        ovm = np.zeros((P, TMK), np.float16)
        ovmT = np.zeros((P, TMK), np.float16)
        for t in range(NT):
            CLt, CHt = CLs[t], CHs[t]
            mt = t_c == t
            s_t, p_t = s_c[mt], p_c[mt]
            lo_t = s_t < HALF
            for half in (0, 1):
                if half == 0:
                    ss, pp = s_t[lo_t], p_t[lo_t]
                    nch, koff = CLt, 0
                else:
                    ss, pp = s_t[~lo_t] - HALF, p_t[~lo_t]
                    nch, koff = CHt, CLt
                if nch == 0:
                    continue
                iv = np.zeros(nch * P, np.int16)
                ne = len(ss)
                iv[:ne] = ss
                jj = np.arange(ne)
                kk, sl = jj // P, jj % P
                vmk[sl, ovo[t] + koff + kk] = 1.0
                ovm[sl, omo[t] + (koff + kk) * P + pp] = 1.0
                ovmT[pp, omo[t] + (koff + kk) * P + sl] = 1.0
                pk = pack16(iv)
                if half == 0:
                    ilo[:, olo[t]:olo[t] + nch * 8] = pk
                else:
                    ihi[:, ohi[t]:ohi[t] + nch * 8] = pk

        adrows = (c * NPC + np.arange(NT)[None, :] * P
                  + np.arange(P)[:, None]).astype(np.int32)
        np.clip(adrows, 0, N + P - 1, out=adrows)
        per_core.append({
            "idxlo": np.ascontiguousarray(ilo),
            "idxhi": np.ascontiguousarray(ihi),
            "vmask": np.ascontiguousarray(vmk),
            "ovmask": np.ascontiguousarray(ovm),
            "ovmaskT": np.ascontiguousarray(ovmT),
            "adrows": np.ascontiguousarray(adrows),
        })

    # block-diagonal attention projectors: as1 = h1 @ asrc_blk
    asrc_blk = np.zeros((cfg.C1, H), np.float32)
    adst_blk = np.zeros((cfg.C1, H), np.float32)
    for h in range(H):
        asrc_blk[h * HID:(h + 1) * HID, h] = a_src1[h]
        adst_blk[h * HID:(h + 1) * HID, h] = a_dst1[h]

    b1row = np.zeros((1, cfg.C1 + 2 * H), np.float32)
    b1row[0, :cfg.C1] = b1
    b2row = np.zeros((1, cfg.OUT + 2), np.float32)
    b2row[0, :cfg.OUT] = b2

    common = {
        "xT": np.ascontiguousarray(np.asarray(x, np.float16).T),
        "W1h": np.ascontiguousarray(np.asarray(W1, np.float16)),
        "W1T": np.ascontiguousarray(np.asarray(W1, np.float32).T),
        "asrcblk": asrc_blk, "adstblk": adst_blk, "b1row": b1row,
        "W2h": np.ascontiguousarray(np.asarray(W2, np.float16)),
        "W2T": np.ascontiguousarray(np.asarray(W2, np.float32).T),
        "a2src": np.ascontiguousarray(
            np.asarray(a_src2, np.float32).reshape(-1, 1)),
        "a2dst": np.ascontiguousarray(
            np.asarray(a_dst2, np.float32).reshape(-1, 1)),
        "b2row": b2row,
        "identh": np.eye(P, dtype=np.float16),
        "onesrow": np.ones((1, P), np.float32),
    }
    return plan, common, per_core


def _build(cfg, plan):
    N, H, HID, C1 = cfg.N, cfg.H, cfg.HID, cfg.C1
    NT, NPC, NNT = cfg.NT, cfg.NPC, cfg.NNT
    C2 = cfg.OUT
    CLs, CHs, Ss, SMAX = plan.CLs, plan.CHs, plan.Ss, plan.SMAX
    olo, ohi, ovo, omo = plan.olo, plan.ohi, plan.ovo, plan.omo
    G1W, G2W = cfg.G1W, cfg.G2W
    G1C = C1 + 2 * H                       # 136 used cols in G1 rows
    G2C = C2 + 2                           # 34 used cols in G2 rows

    nc = bacc.Bacc("TRN2", target_bir_lowering=False, debug=False,
                   num_devices=cfg.NC)

    def din(name, shape, dt=F32):
        return nc.dram_tensor(name, shape, dt, kind="ExternalInput").ap()

    xT = din("xT", [P, N], F16)
    W1h = din("W1h", [P, C1], F16)
    W1T = din("W1T", [C1, P])
    asrcblk = din("asrcblk", [C1, H])
    adstblk = din("adstblk", [C1, H])
    b1row = din("b1row", [1, G1C])
    W2h = din("W2h", [C1, C2], F16)
    W2T = din("W2T", [C2, C1])
    a2src = din("a2src", [C2, 1])
    a2dst = din("a2dst", [C2, 1])
    b2row = din("b2row", [1, G2C])
    identh = din("identh", [P, P], F16)
    onesrow = din("onesrow", [1, P])
    idxlo = din("idxlo", [P, olo[NT]], I16)
    idxhi = din("idxhi", [P, ohi[NT]], I16)
    vmask = din("vmask", [P, ovo[NT]], F16)
    ovmask = din("ovmask", [P, omo[NT]], F16)
    ovmaskT = din("ovmaskT", [P, omo[NT]], F16)
    adrows = din("adrows", [P, NT], I32)

    out = nc.dram_tensor("out", [NT * P, C2], F32, kind="ExternalOutput").ap()

    G1 = nc.dram_tensor("G1", [N + P, G1W], F16).ap()
    G2 = nc.dram_tensor("G2", [N + P, G2W], F16).ap()
    G2c = nc.dram_tensor("G2c", [NPC, G2C], F16).ap()
    TSPLIT = (24, 42)                      # allgather after these tiles
    ROW0 = (0, TSPLIT[0] * P, TSPLIT[1] * P, NPC)
    G2cfs = [nc.dram_tensor(f"G2cf{j}",
                            [cfg.NC * (ROW0[j + 1] - ROW0[j]), G2C], F16,
                            addr_space="Shared").ap() for j in range(3)]
    PRE = nc.dram_tensor("PRE", [NT * P, C2], F32).ap()

    with tile.TileContext(nc) as tc:
        with tc.tile_pool(name="const", bufs=1) as const:
            # ---- constants / fused weight tables --------------------------
            with tc.tile_pool(name="cpsum", bufs=1, space="PSUM") as cpsum:
                w1ext = const.tile([P, G1C], F16)   # [W1 | W1@Asrc | W1@Adst]
                nc.sync.dma_start(out=w1ext[:, 0:C1], in_=W1h)
                w1t = const.tile([P, P], F32)
                nc.sync.dma_start(out=w1t[:], in_=W1T)
                ablk = const.tile([P, 2 * H], F32)
                nc.sync.dma_start(out=ablk[:, 0:H], in_=asrcblk)
                nc.sync.dma_start(out=ablk[:, H:2 * H], in_=adstblk)
                pw = cpsum.tile([P, 2 * H], F32, space="PSUM")
                nc.tensor.matmul(pw[:], lhsT=w1t[:], rhs=ablk[:], start=True,
                                 stop=True)
                nc.vector.tensor_copy(out=w1ext[:, C1:C1 + 2 * H], in_=pw[:])

                w2ext = const.tile([P, G2C], F16)   # [W2 | W2@a2s | W2@a2d]
                nc.sync.dma_start(out=w2ext[:, 0:C2], in_=W2h)
                w2t = const.tile([C2, C1], F32)
                nc.sync.dma_start(out=w2t[:], in_=W2T)
                a2 = const.tile([C2, 2], F32)
                nc.sync.dma_start(out=a2[:, 0:1], in_=a2src)
                nc.sync.dma_start(out=a2[:, 1:2], in_=a2dst)
                pw2 = cpsum.tile([P, 2], F32, space="PSUM")
                nc.tensor.matmul(pw2[:], lhsT=w2t[:], rhs=a2[:], start=True,
                                 stop=True)
                nc.vector.tensor_copy(out=w2ext[:, C2:C2 + 2], in_=pw2[:])

                # broadcast bias rows to all 128 partitions (ones @ brow)
                b1sb = const.tile([1, G1C], F32)
                nc.sync.dma_start(out=b1sb[:], in_=b1row)
                b2sb = const.tile([1, G2C], F32)
                nc.sync.dma_start(out=b2sb[:], in_=b2row)
                onesb = const.tile([1, P], F32)
                nc.sync.dma_start(out=onesb[:], in_=onesrow)
                b1rep = const.tile([P, G1C], F32)
                pb1 = cpsum.tile([P, G1C], F32, space="PSUM")
                nc.tensor.matmul(pb1[:], lhsT=onesb[:], rhs=b1sb[:],
                                 start=True, stop=True)
                nc.vector.tensor_copy(out=b1rep[:], in_=pb1[:])
                b2rep = const.tile([P, G2C], F32)
                pb2 = cpsum.tile([P, G2C], F32, space="PSUM")
                nc.tensor.matmul(pb2[:], lhsT=onesb[:], rhs=b2sb[:],
                                 start=True, stop=True)
                nc.vector.tensor_copy(out=b2rep[:], in_=pb2[:])

                idsb = const.tile([P, P], F16)
                nc.sync.dma_start(out=idsb[:], in_=identh)
                adr = const.tile([P, NT], I32)
                nc.sync.dma_start(out=adr[:], in_=adrows)
                # prefetch all per-tile gather indices / validity masks
                ilosb = const.tile([P, olo[NT]], I16)
                nc.sync.dma_start(out=ilosb[:], in_=idxlo)
                ihisb = const.tile([P, ohi[NT]], I16)
                nc.sync.dma_start(out=ihisb[:], in_=idxhi)
                vmsb = const.tile([P, ovo[NT]], F16)
                nc.sync.dma_start(out=vmsb[:], in_=vmask)
                adt2sb = const.tile([P, NT], F16)
                nc.vector.memset(adt2sb[:], 0.0)

            # SBUF pools stay open across phases so later phases' tiles
            # never WAR-collide with earlier phases' addresses (lets phase-B
            # hi gathers start while phase A still writes the lo table half).
            sb_pools = tc.tile_pool(name="pa", bufs=3), \
                tc.tile_pool(name="pbig", bufs=4), \
                tc.tile_pool(name="pxx", bufs=2), \
                tc.tile_pool(name="pmed", bufs=2), \
                tc.tile_pool(name="pmsk", bufs=2), \
                tc.tile_pool(name="pdig", bufs=4), \
                tc.tile_pool(name="pdxx", bufs=2), \
                tc.tile_pool(name="pdmd", bufs=2), \
                tc.tile_pool(name="pdmk", bufs=2)
            import contextlib
            _stack = contextlib.ExitStack()
            pa, pbig, pxx, pmed, pmsk, pdig, pdxx, pdmd, pdmk = (
                _stack.enter_context(p) for p in sb_pools)
            # pre-clean gather buffers (garbage SBUF could decode as NaN f16;
            # NaN survives the 0-weight mask since 0*NaN=NaN)
            for _ in range(4):
                vgz = pbig.tile([P, SMAX * G1W], F16, tag="vg")
                nc.vector.memset(vgz[:], 0.0)
                vgz2 = pdig.tile([P, SMAX * G2W], F16, tag="vg2")
                nc.vector.memset(vgz2[:], 0.0)

            # ---- phase A: G1 rows -----------------------------------------
            with (
                tc.tile_pool(name="pap", bufs=4, space="PSUM") as pap,
            ):
                zz = pa.tile([P, G1W], F16, tag="zz")
                nc.vector.memset(zz[:], 0.0)
                nc.sync.dma_start(out=G1[N:N + P, :], in_=zz[:])
                nc.sync.dma_start(out=G2[N:N + P, :], in_=zz[:, 0:G2W])
                GA = 8
                NHI = HALF // P            # first lo tile index
                groups = ([(i, min(GA, NNT - i))
                           for i in range(NHI, NNT, GA)]
                          + [(i, min(GA, NHI - i))
                             for i in range(0, NHI, GA)])
                for i0, gg in groups:
                    wid = (gg - 1) * P + (P if i0 + gg < NNT else cfg.LASTN)
                    xt = pa.tile([P, GA * P], F16, tag="xt")
                    nc.sync.dma_start(out=xt[:, 0:wid],
                                      in_=xT[:, i0 * P:i0 * P + wid])
                    g1h = pa.tile([P, GA * G1C], F16, tag="g1h")
                    for g in range(gg):
                        nn = P if i0 + g < NNT - 1 else cfg.LASTN
                        ps = pap.tile([P, G1C], F32, space="PSUM", tag="ps")
                        nc.tensor.matmul(ps[:nn, :],
                                         lhsT=xt[:, g * P:g * P + nn],
                                         rhs=w1ext[:], start=True, stop=True)
                        nc.vector.tensor_tensor(
                            out=g1h[:nn, g * G1C:(g + 1) * G1C],
                            in0=ps[:nn, :], in1=b1rep[:nn, :], op=OP.add)
                    if gg == GA and wid == GA * P:
                        nc.scalar.dma_start(
                            out=G1[i0 * P:(i0 + GA) * P, 0:G1C]
                                .rearrange("(g p) c -> p g c", g=GA),
                            in_=g1h[:].rearrange("p (g c) -> p g c", g=GA))
                    else:
                        for g in range(gg):
                            nn = P if i0 + g < NNT - 1 else cfg.LASTN
                            nc.scalar.dma_start(
                                out=G1[(i0 + g) * P:(i0 + g) * P + nn, 0:G1C],
                                in_=g1h[:nn, g * G1C:(g + 1) * G1C])

            # ---- phase B: layer-1 edges + layer-2 row build ---------------
            with (
                tc.tile_pool(name="pbp", bufs=2, space="PSUM") as pbp,
                tc.tile_pool(name="pbpa", bufs=2, space="PSUM") as pbpa,
                tc.tile_pool(name="pcp", bufs=1, space="PSUM") as pcp,
                tc.tile_pool(name="pcpt", bufs=1, space="PSUM") as pcpt,
            ):
                for t in range(NT):
                    ndst = P if t < NT - 1 else cfg.LAST
                    CLt, CHt, St = CLs[t], CHs[t], Ss[t]
                    vg = pbig.tile([P, SMAX * G1W], F16, tag="vg")
                    vg3 = vg[:].rearrange("p (k c) -> p k c", c=G1W)
                    if CHt:
                        nc.gpsimd.dma_gather(
                            out_ap=vg3[:, CLt:St, :], in_ap=G1[HALF:N + P, :],
                            idxs_ap=ihisb[:, ohi[t]:ohi[t] + CHt * 8],
                            num_idxs=CHt * P, num_idxs_reg=CHt * P,
                            elem_size=G1W, single_packet=False)
                    if CLt:
                        nc.gpsimd.dma_gather(
                            out_ap=vg3[:, 0:CLt, :],
                            in_ap=G1[0:min(HALF, N + P), :],
                            idxs_ap=ilosb[:, olo[t]:olo[t] + CLt * 8],
                            num_idxs=CLt * P, num_idxs_reg=CLt * P,
                            elem_size=G1W, single_packet=False)
                    # alpha_dst values of this tile's dsts, per partition
                    adt = pmed.tile([P, H], F16, tag="adt")
                    nc.gpsimd.indirect_dma_start(
                        out=adt[:], out_offset=None, in_=G1,
                        in_offset=IndirectOffsetOnAxis(ap=adr[:, t:t + 1],
                                                       axis=0),
                        element_offset=C1 + H)
                    msk = pmsk.tile([P, SMAX * P], F16, tag="msk")
                    nc.sync.dma_start(
                        out=msk[:, 0:St * P],
                        in_=ovmask[:, omo[t]:omo[t] + St * P])
                    mskT = pmsk.tile([P, SMAX * P], F16, tag="mskT")
                    nc.scalar.dma_start(
                        out=mskT[:, 0:St * P],
                        in_=ovmaskT[:, omo[t]:omo[t] + St * P])

                    # per-slot alpha_dst for every chunk: maskT @ adt
                    adp = pbpa.tile([P, SMAX * H], F32, space="PSUM",
                                    tag="adp")
                    for k in range(St):
                        nc.tensor.matmul(
                            adp[:, k * H:(k + 1) * H],
                            lhsT=mskT[:, k * P:(k + 1) * P],
                            rhs=adt[:], start=True, stop=True)
                    alp = pmed.tile([P, SMAX * H], F16, tag="alp")
                    alp3 = alp[:].rearrange("p (k h) -> p k h", h=H)
                    nc.vector.tensor_tensor(
                        out=alp3[:, 0:St, :],
                        in0=vg3[:, 0:St, C1:C1 + H],
                        in1=adp[:].rearrange("p (k h) -> p k h", h=H)
                            [:, 0:St, :],
                        op=OP.add)
                    # e = exp(lrelu(alpha)) * vmask
                    asc = pmed.tile([P, SMAX * H], F16, tag="asc")
                    nc.vector.tensor_scalar(out=asc[:, 0:St * H],
                                            in0=alp[:, 0:St * H],
                                            scalar1=cfg.NEG, scalar2=None,
                                            op0=OP.mult)
                    lrl = pmed.tile([P, SMAX * H], F16, tag="lrl")
                    nc.vector.tensor_tensor(out=lrl[:, 0:St * H],
                                            in0=alp[:, 0:St * H],
                                            in1=asc[:, 0:St * H], op=OP.max)
                    ee = pmed.tile([P, SMAX * H], F32, tag="ee")
                    nc.scalar.activation(out=ee[:, 0:St * H],
                                         in_=lrl[:, 0:St * H], func=AF.Exp)
                    eeh = pmed.tile([P, SMAX * H], F16, tag="eeh")
                    nc.vector.tensor_tensor(
                        out=eeh[:, 0:St * H].rearrange("p (k h) -> p k h",
                                                       h=H),
                        in0=ee[:, 0:St * H].rearrange("p (k h) -> p k h",
                                                      h=H),
                        in1=vmsb[:, ovo[t]:ovo[t] + St]
                            .rearrange("p (k o) -> p k o", o=1)
                            .to_broadcast([P, St, H]),
                        op=OP.mult)
                    eeh3 = eeh[:].rearrange("p (k h) -> p k h", h=H)
                    # rhs = [e*h | e]
                    xx = pxx.tile([P, SMAX * (C1 + H)], F16, tag="xx")
                    xx3 = xx[:].rearrange("p (k c) -> p k c", c=C1 + H)
                    nc.vector.tensor_copy(out=xx3[:, 0:St, C1:C1 + H],
                                          in_=eeh3[:, 0:St, :])
                    nc.vector.tensor_tensor(
                        out=xx3[:, 0:St, 0:C1].rearrange(
                            "p k (h c) -> p k h c", c=HID),
                        in0=vg3[:, 0:St, 0:C1].rearrange(
                            "p k (h c) -> p k h c", c=HID),
                        in1=eeh[:, 0:St * H].rearrange(
                            "p (k h o) -> p k h o", h=H, o=1)
                            .to_broadcast([P, St, H, HID]),
                        op=OP.mult)
                    ps = pbp.tile([P, C1 + H], F32, space="PSUM", tag="ps")
                    for k in range(St):
                        nc.tensor.matmul(ps[:], lhsT=msk[:, k * P:(k + 1) * P],
                                         rhs=xx3[:, k, :],
                                         start=(k == 0), stop=(k == St - 1))
                    rec = pmed.tile([P, H], F32, tag="rec")
                    nc.vector.reciprocal(out=rec[:ndst, :],
                                         in_=ps[:ndst, C1:C1 + H])
                    o1 = pmed.tile([P, C1], F16, tag="o1")
                    if ndst < P:
                        nc.vector.memset(o1[:], 0.0)
                    for h in range(H):
                        nc.vector.tensor_scalar(
                            out=o1[:ndst, h * HID:(h + 1) * HID],
                            in0=ps[:ndst, h * HID:(h + 1) * HID],
                            scalar1=rec[:ndst, h:h + 1], scalar2=0.0,
                            op0=OP.mult, op1=OP.max)
                    # layer-2 row build: transpose + project
                    tp = pcpt.tile([P, P], F16, space="PSUM", tag="tp")
                    nc.tensor.transpose(out=tp[:], in_=o1[:],
                                        identity=idsb[:])
                    o1t = pmed.tile([P, P], F16, tag="o1t")
                    nc.vector.tensor_copy(out=o1t[:], in_=tp[:])
                    hp = pcp.tile([P, G2C], F32, space="PSUM", tag="hp")
                    nc.tensor.matmul(hp[:], lhsT=o1t[:], rhs=w2ext[:],
                                     start=True, stop=True)
                    g2h = pmed.tile([P, G2C], F16, tag="g2h")
                    nc.vector.tensor_tensor(out=g2h[:ndst, :],
                                            in0=hp[:ndst, :],
                                            in1=b2rep[:ndst, :], op=OP.add)
                    nc.vector.tensor_copy(out=adt2sb[:ndst, t:t + 1],
                                          in_=g2h[:ndst, C2 + 1:C2 + 2])
                    nc.sync.dma_start(out=G2c[t * P:t * P + ndst, :],
                                      in_=g2h[:ndst, :])
                    if t + 1 in TSPLIT:
                        # partial table replication overlaps the rest of B
                        j = TSPLIT.index(t + 1)
                        r0, r1 = ROW0[j], ROW0[j + 1]
                        w = r1 - r0
                        nc.gpsimd.collective_compute(
                            "AllGather", OP.bypass,
                            replica_groups=[list(range(cfg.NC))],
                            ins=[G2c[r0:r1, :]], outs=[G2cfs[j]])
                        for cc in range(cfg.NC):
                            nc.sync.dma_start(
                                out=G2[cc * NPC + r0:cc * NPC + r1, 0:G2C],
                                in_=G2cfs[j][cc * w:(cc + 1) * w, :])

            r0, r1 = ROW0[2], ROW0[3]
            w = r1 - r0
            nc.gpsimd.collective_compute(
                "AllGather", OP.bypass,
                replica_groups=[list(range(cfg.NC))],
                ins=[G2c[r0:r1, :]], outs=[G2cfs[2]])
            for cc in range(cfg.NC):
                nc.sync.dma_start(
                    out=G2[cc * NPC + r0:cc * NPC + r1, 0:G2C],
                    in_=G2cfs[2][cc * w:(cc + 1) * w, :])

            # ---- phase D: layer-2 edge aggregation (1 head) ---------------
            with (
                tc.tile_pool(name="pdp", bufs=2, space="PSUM") as pdp,
                tc.tile_pool(name="pdpa", bufs=2, space="PSUM") as pdpa,
            ):
                for t in range(NT):
                    ndst = P if t < NT - 1 else cfg.LAST
                    CLt, CHt, St = CLs[t], CHs[t], Ss[t]
                    vg = pdig.tile([P, SMAX * G2W], F16, tag="vg2")
                    vg3 = vg[:].rearrange("p (k c) -> p k c", c=G2W)
                    if CLt:
                        nc.gpsimd.dma_gather(
                            out_ap=vg3[:, 0:CLt, :],
                            in_ap=G2[0:min(HALF, N + P), :],
                            idxs_ap=ilosb[:, olo[t]:olo[t] + CLt * 8],
                            num_idxs=CLt * P, num_idxs_reg=CLt * P,
                            elem_size=G2W, single_packet=False)
                    if CHt:
                        nc.gpsimd.dma_gather(
                            out_ap=vg3[:, CLt:St, :], in_ap=G2[HALF:N + P, :],
                            idxs_ap=ihisb[:, ohi[t]:ohi[t] + CHt * 8],
                            num_idxs=CHt * P, num_idxs_reg=CHt * P,
                            elem_size=G2W, single_packet=False)
                    adt = adt2sb[:, t:t + 1]
                    msk = pdmk.tile([P, SMAX * P], F16, tag="msk2")
                    nc.sync.dma_start(
                        out=msk[:, 0:St * P],
                        in_=ovmask[:, omo[t]:omo[t] + St * P])
                    mskT = pdmk.tile([P, SMAX * P], F16, tag="mskT2")
                    nc.scalar.dma_start(
                        out=mskT[:, 0:St * P],
                        in_=ovmaskT[:, omo[t]:omo[t] + St * P])

                    adp = pdpa.tile([P, SMAX], F32, space="PSUM", tag="adp2")
                    for k in range(St):
                        nc.tensor.matmul(
                            adp[:, k:k + 1],
                            lhsT=mskT[:, k * P:(k + 1) * P],
                            rhs=adt, start=True, stop=True)
                    alp = pdmd.tile([P, SMAX], F16, tag="alp2")
                    alp3 = alp[:].rearrange("p (k o) -> p k o", o=1)
                    nc.vector.tensor_tensor(
                        out=alp3[:, 0:St, :],
                        in0=vg3[:, 0:St, C2:C2 + 1],
                        in1=adp[:].rearrange("p (k o) -> p k o", o=1)
                            [:, 0:St, :],
                        op=OP.add)
                    asc = pdmd.tile([P, SMAX], F16, tag="asc2")
                    nc.vector.tensor_scalar(out=asc[:, 0:St],
                                            in0=alp[:, 0:St],
                                            scalar1=cfg.NEG, scalar2=None,
                                            op0=OP.mult)
                    lrl = pdmd.tile([P, SMAX], F16, tag="lrl2")
                    nc.vector.tensor_tensor(out=lrl[:, 0:St],
                                            in0=alp[:, 0:St],
                                            in1=asc[:, 0:St], op=OP.max)
                    ee = pdmd.tile([P, SMAX], F32, tag="ee2")
                    nc.scalar.activation(out=ee[:, 0:St], in_=lrl[:, 0:St],
                                         func=AF.Exp)
                    eeh = pdmd.tile([P, SMAX], F16, tag="eeh2")
                    nc.vector.tensor_tensor(out=eeh[:, 0:St],
                                            in0=ee[:, 0:St],
                                            in1=vmsb[:, ovo[t]:ovo[t] + St],
                                            op=OP.mult)
                    xx = pdxx.tile([P, SMAX * (C2 + 1)], F16, tag="xx2")
                    xx3 = xx[:].rearrange("p (k c) -> p k c", c=C2 + 1)
                    nc.vector.tensor_copy(
                        out=xx3[:, 0:St, C2:C2 + 1],
                        in_=eeh[:, 0:St].rearrange("p (k o) -> p k o", o=1))
                    nc.vector.tensor_tensor(
                        out=xx3[:, 0:St, 0:C2],
                        in0=vg3[:, 0:St, 0:C2],
                        in1=eeh[:, 0:St].rearrange("p (k o) -> p k o", o=1)
                            .to_broadcast([P, St, C2]),
                        op=OP.mult)
                    ps = pdp.tile([P, C2 + 1], F32, space="PSUM", tag="ps2")
                    for k in range(St):
                        nc.tensor.matmul(ps[:], lhsT=msk[:, k * P:(k + 1) * P],
                                         rhs=xx3[:, k, :],
                                         start=(k == 0), stop=(k == St - 1))
                    rec = pdmd.tile([P, 1], F32, tag="rec2")
                    nc.vector.reciprocal(out=rec[:ndst, :],
                                         in_=ps[:ndst, C2:C2 + 1])
                    o2 = pdmd.tile([P, C2], F32, tag="o2")
                    nc.vector.tensor_scalar(out=o2[:ndst, :],
                                            in0=ps[:ndst, 0:C2],
                                            scalar1=rec[:ndst, :],
                                            scalar2=None, op0=OP.mult)
                    nc.sync.dma_start(out=PRE[t * P:t * P + ndst, :],
                                      in_=o2[:ndst, :])

            _stack.close()

            # ---- phase E: one sigmoid sweep -------------------------------
            with tc.tile_pool(name="pe", bufs=2) as pe:
                FW = NT * C2
                pre_f = PRE.rearrange("(a b) c -> a (b c)", a=P)
                out_f = out.rearrange("(a b) c -> a (b c)", a=P)
                pei = pe.tile([P, FW], F32, tag="pei")
                nc.sync.dma_start(out=pei[:], in_=pre_f)
                peo = pe.tile([P, FW], F32, tag="peo")
                nc.scalar.activation(out=peo[:], in_=pei[:], func=AF.Sigmoid)
                nc.sync.dma_start(out=out_f, in_=peo[:])

    nc.compile()
    return nc


_CACHE: dict = {}


def _get_module(cfg, plan):
    key = (cfg.N, cfg.E, plan.CLs, plan.CHs)
    if key not in _CACHE:
        _CACHE[key] = _build(cfg, plan)
    return _CACHE[key]


def _run(cfg, inputs, trace=False):
    plan, common, per_core = _prep_host(
        cfg, inputs["x"], inputs["edge_index"], inputs["W1"],
        inputs["a_src1"], inputs["a_dst1"], inputs["b1"], inputs["W2"],
        inputs["a_src2"], inputs["a_dst2"], inputs["b2"])
    nc = _get_module(cfg, plan)
    in_maps = [dict(common, **pc) for pc in per_core]
    res = run_bass_kernel_spmd(nc, in_maps, core_ids=list(range(cfg.NC)),
                               trace=trace)
    shards = [np.asarray(res.results[c]["out"])[:cfg.NPC]
              for c in range(cfg.NC)]
    full = np.concatenate(shards, axis=0).astype(np.float32)
    return (full, res) if trace else full


def kernel(**inputs) -> np.ndarray:
    cfg = GATConfig(n=50000, e=800000)
    return _run(cfg, inputs)


# revision 15
# speedup vs baseline: 1.1050x; 1.0742x over previous
"""GAT 2-layer propagation kernel for Trainium2, 8 NeuronCores (SPMD).

Strategy (edge-parallel, dst-node-range sharded across 8 cores):
  - Core c owns dst nodes [c*6250, (c+1)*6250); edges (with self-loops) go to
    the core owning their dst, so each core emits its contiguous output rows
    and no reduction collective is needed.
  - Per layer a DRAM gather table holds one fp16 row per node:
      G1[n] = [h1 x128 | as1 x4 | ad1 x4 | pad]   (512 B rows)
      G2[n] = [h2 x32  | as2    | ad2    | pad]   (256 B rows)
    (h carries the layer bias folded in: softmax weights sum to 1, so adding
    b to every value row adds b to the output.)
  - Per 128-dst tile, edges are packed DENSELY into chunks of 128 slots
    (partition dim), lo-src chunks then hi-src chunks (dma_gather indices
    are int16, so the node table is addressed in two halves).  Dense packing
    needs ~19 chunks/tile vs ~21 for the aligned layout, directly shrinking
    GpSimd descriptor generation (the serial bottleneck, ~8ns/row).  Unused
    tail slots gather row 0; the validity mask zeroes their weight.
  - Each chunk k has a host-precomputed [slot, dst] 0/1 selection mask (and
    its transpose).  maskT @ adt recovers per-slot alpha_dst; mask is the
    stationary operand of the per-chunk segment-sum matmul, which
    accumulates [sum e*h | sum e] in fp32 PSUM across chunks; the softmax
    division happens once per dst at the end.
  - e = exp(leakyrelu(as+ad)) needs no max-subtraction (|alpha| <= ~6 here);
    exp runs in fp32 so stale-slot garbage stays finite, and the validity
    mask zeroes it before the fp16 value multiply.
  - Layer-2 table rows are built inline as each layer-1 output tile
    finishes; an AllGather + relayout replicates the table. The final
    sigmoid runs as one deferred sweep so ACT's activation table stays on
    Exp during edge phases.
"""

import numpy as np

import concourse.bacc as bacc
import concourse.tile as tile
from concourse import mybir
from concourse.bass import IndirectOffsetOnAxis
from concourse.bass_utils import run_bass_kernel_spmd

F32 = mybir.dt.float32
F16 = mybir.dt.float16
I32 = mybir.dt.int32
I16 = mybir.dt.int16
AF = mybir.ActivationFunctionType
OP = mybir.AluOpType

P = 128
HALF = 32768            # int16-addressable rows per dma_gather call


class GATConfig:
    def __init__(self, n, e, in_dim=128, hid=32, heads=4, out_dim=32,
                 neg_slope=0.2, n_cores=8):
        assert in_dim == P and heads * hid == P
        self.N, self.E = n, e
        self.HID = hid
        self.H = heads
        self.OUT = out_dim
        self.NEG = neg_slope
        self.NC = n_cores
        assert n % n_cores == 0
        self.NPC = n // n_cores
        self.NT = (self.NPC + P - 1) // P
        self.LAST = self.NPC - (self.NT - 1) * P
        self.C1 = heads * hid                 # 128
        self.G1W = 256                        # fp16 els/row: h|as|ad|pad
        self.G2W = 128
        self.NNT = (n + P - 1) // P
        self.LASTN = n - (self.NNT - 1) * P


class EdgePlan:
    """Dense chunk structure: per-tile chunk counts (max over cores)."""


def _prep_host(cfg, x, edge_index, W1, a_src1, a_dst1, b1, W2, a_src2,
               a_dst2, b2):
    N, H, HID = cfg.N, cfg.H, cfg.HID
    NPC, NT, NC = cfg.NPC, cfg.NT, cfg.NC

    src = np.concatenate([np.asarray(edge_index[0], dtype=np.int64),
                          np.arange(N, dtype=np.int64)])
    dst = np.concatenate([np.asarray(edge_index[1], dtype=np.int64),
                          np.arange(N, dtype=np.int64)])
    order = np.argsort(dst, kind="stable")
    src, dst = src[order], dst[order]

    core_of = dst // NPC
    tile_of = (dst % NPC) // P
    part_of = (dst % NPC) % P
    # permuted table row: node n lives at (n%128)*512 + n//128, so the
    # int16 lo/hi table halves are the p<64 / p>=64 partition slices
    psrc = (src % P) * 512 + src // P
    is_hi = psrc >= HALF

    nlo = np.zeros((NC, NT), np.int64)
    nhi = np.zeros((NC, NT), np.int64)
    np.add.at(nlo, (core_of[~is_hi], tile_of[~is_hi]), 1)
    np.add.at(nhi, (core_of[is_hi], tile_of[is_hi]), 1)

    CLs = np.ceil(nlo.max(axis=0) / P).astype(int)     # per-tile, all cores
    CHs = np.ceil(nhi.max(axis=0) / P).astype(int)
    Ss = CLs + CHs
    plan = EdgePlan()
    plan.CLs, plan.CHs = tuple(int(v) for v in CLs), tuple(int(v) for v in CHs)
    plan.Ss = tuple(int(v) for v in Ss)
    plan.SMAX = int(Ss.max())
    olo = np.concatenate([[0], np.cumsum(CLs * 8)])    # idx col offsets
    ohi = np.concatenate([[0], np.cumsum(CHs * 8)])
    ovo = np.concatenate([[0], np.cumsum(Ss)])         # vmask col offsets
    omo = np.concatenate([[0], np.cumsum(Ss * P)])     # mask col offsets
    plan.olo, plan.ohi = tuple(olo.tolist()), tuple(ohi.tolist())
    plan.ovo, plan.omo = tuple(ovo.tolist()), tuple(omo.tolist())
    TLO, THI, TVM, TMK = olo[-1], ohi[-1], ovo[-1], omo[-1]

    def pack16(vals):
        # vals: [n_chunks*P] int16 in slot order j=k*128+p; idx j lives at
        # [j%16, j//16], replicated across the 8 stripes of 16 partitions.
        a = vals.reshape(-1, 16).T.astype(np.int16)   # [16, n/16]
        return np.tile(a, (8, 1))

    per_core = []
    for c in range(NC):
        m = core_of == c
        s_c = psrc[m].astype(np.int64)
        t_c = tile_of[m]
        p_c = part_of[m]
        ilo = np.zeros((P, TLO), np.int16)
        ihi = np.zeros((P, THI), np.int16)
        vmk = np.zeros((P, TVM), np.float16)
        ovm = np.zeros((P, TMK), np.float16)
        ovmT = np.zeros((P, TMK), np.float16)
        for t in range(NT):
            CLt, CHt = CLs[t], CHs[t]
            mt = t_c == t
            s_t, p_t = s_c[mt], p_c[mt]
            lo_t = s_t < HALF
            for half in (0, 1):
                if half == 0:
                    ss, pp = s_t[lo_t], p_t[lo_t]
                    nch, koff = CLt, 0
                else:
                    ss, pp = s_t[~lo_t] - HALF, p_t[~lo_t]
                    nch, koff = CHt, CLt
                if nch == 0:
                    continue
                iv = np.zeros(nch * P, np.int16)
                ne = len(ss)
                iv[:ne] = ss
                jj = np.arange(ne)
                kk, sl = jj // P, jj % P
                vmk[sl, ovo[t] + koff + kk] = 1.0
                ovm[sl, omo[t] + (koff + kk) * P + pp] = 1.0
                ovmT[pp, omo[t] + (koff + kk) * P + sl] = 1.0
                pk = pack16(iv)
                if half == 0:
                    ilo[:, olo[t]:olo[t] + nch * 8] = pk
                else:
                    ihi[:, ohi[t]:ohi[t] + nch * 8] = pk

        nrow = (c * NPC + np.arange(NT)[None, :] * P
                + np.arange(P)[:, None])
        np.clip(nrow, 0, N - 1, out=nrow)
        adrows = ((nrow % P) * 512 + nrow // P).astype(np.int32)
        per_core.append({
            "idxlo": np.ascontiguousarray(ilo),
            "idxhi": np.ascontiguousarray(ihi),
            "vmask": np.ascontiguousarray(vmk),
            "ovmask": np.ascontiguousarray(ovm),
            "ovmaskT": np.ascontiguousarray(ovmT),
            "adrows": np.ascontiguousarray(adrows),
        })

    # block-diagonal attention projectors: as1 = h1 @ asrc_blk
    asrc_blk = np.zeros((cfg.C1, H), np.float32)
    adst_blk = np.zeros((cfg.C1, H), np.float32)
    for h in range(H):
        asrc_blk[h * HID:(h + 1) * HID, h] = a_src1[h]
        adst_blk[h * HID:(h + 1) * HID, h] = a_dst1[h]

    b1row = np.zeros((1, cfg.C1 + 2 * H), np.float32)
    b1row[0, :cfg.C1] = b1
    b2row = np.zeros((1, cfg.OUT + 2), np.float32)
    b2row[0, :cfg.OUT] = b2

    common = {
        "xT": np.ascontiguousarray(np.asarray(x, np.float16).T),
        "W1h": np.ascontiguousarray(np.asarray(W1, np.float16)),
        "W1T": np.ascontiguousarray(np.asarray(W1, np.float32).T),
        "asrcblk": asrc_blk, "adstblk": adst_blk, "b1row": b1row,
        "W2h": np.ascontiguousarray(np.asarray(W2, np.float16)),
        "W2T": np.ascontiguousarray(np.asarray(W2, np.float32).T),
        "a2src": np.ascontiguousarray(
            np.asarray(a_src2, np.float32).reshape(-1, 1)),
        "a2dst": np.ascontiguousarray(
            np.asarray(a_dst2, np.float32).reshape(-1, 1)),
        "b2row": b2row,
        "identh": np.eye(P, dtype=np.float16),
        "onesrow": np.ones((1, P), np.float32),
    }
    return plan, common, per_core


def _build(cfg, plan):
    N, H, HID, C1 = cfg.N, cfg.H, cfg.HID, cfg.C1
    NT, NPC, NNT = cfg.NT, cfg.NPC, cfg.NNT
    C2 = cfg.OUT
    CLs, CHs, Ss, SMAX = plan.CLs, plan.CHs, plan.Ss, plan.SMAX
    olo, ohi, ovo, omo = plan.olo, plan.ohi, plan.ovo, plan.omo
    G1W, G2W = cfg.G1W, cfg.G2W
    G1C = C1 + 2 * H                       # 136 used cols in G1 rows
    G2C = C2 + 2                           # 34 used cols in G2 rows

    nc = bacc.Bacc("TRN2", target_bir_lowering=False, debug=False,
                   num_devices=cfg.NC)

    def din(name, shape, dt=F32):
        return nc.dram_tensor(name, shape, dt, kind="ExternalInput").ap()

    xT = din("xT", [P, N], F16)
    W1h = din("W1h", [P, C1], F16)
    W1T = din("W1T", [C1, P])
    asrcblk = din("asrcblk", [C1, H])
    adstblk = din("adstblk", [C1, H])
    b1row = din("b1row", [1, G1C])
    W2h = din("W2h", [C1, C2], F16)
    W2T = din("W2T", [C2, C1])
    a2src = din("a2src", [C2, 1])
    a2dst = din("a2dst", [C2, 1])
    b2row = din("b2row", [1, G2C])
    identh = din("identh", [P, P], F16)
    onesrow = din("onesrow", [1, P])
    idxlo = din("idxlo", [P, olo[NT]], I16)
    idxhi = din("idxhi", [P, ohi[NT]], I16)
    vmask = din("vmask", [P, ovo[NT]], F16)
    ovmask = din("ovmask", [P, omo[NT]], F16)
    ovmaskT = din("ovmaskT", [P, omo[NT]], F16)
    adrows = din("adrows", [P, NT], I32)

    out = nc.dram_tensor("out", [NT * P, C2], F32, kind="ExternalOutput").ap()

    NROWS = P * 512                        # permuted node-table rows
    G1 = nc.dram_tensor("G1", [NROWS, G1W], F16).ap()
    G2 = nc.dram_tensor("G2", [NROWS, G2W], F16).ap()
    G2c = nc.dram_tensor("G2c", [NPC, G2C], F16).ap()
    TSPLIT = (24, 42)                      # allgather after these tiles
    ROW0 = (0, TSPLIT[0] * P, TSPLIT[1] * P, NPC)
    G2cfs = [nc.dram_tensor(f"G2cf{j}",
                            [cfg.NC * (ROW0[j + 1] - ROW0[j]), G2C], F16,
                            addr_space="Shared").ap() for j in range(3)]
    PRE = nc.dram_tensor("PRE", [NT * P, C2], F32).ap()

    with tile.TileContext(nc) as tc:
        with tc.tile_pool(name="const", bufs=1) as const:
            # ---- constants / fused weight tables --------------------------
            with tc.tile_pool(name="cpsum", bufs=1, space="PSUM") as cpsum:
                w1ext = const.tile([P, G1C], F16)   # [W1 | W1@Asrc | W1@Adst]
                nc.sync.dma_start(out=w1ext[:, 0:C1], in_=W1h)
                w1t = const.tile([P, P], F32)
                nc.sync.dma_start(out=w1t[:], in_=W1T)
                ablk = const.tile([P, 2 * H], F32)
                nc.sync.dma_start(out=ablk[:, 0:H], in_=asrcblk)
                nc.sync.dma_start(out=ablk[:, H:2 * H], in_=adstblk)
                pw = cpsum.tile([P, 2 * H], F32, space="PSUM")
                nc.tensor.matmul(pw[:], lhsT=w1t[:], rhs=ablk[:], start=True,
                                 stop=True)
                nc.vector.tensor_copy(out=w1ext[:, C1:C1 + 2 * H], in_=pw[:])

                w2ext = const.tile([P, G2C], F16)   # [W2 | W2@a2s | W2@a2d]
                nc.sync.dma_start(out=w2ext[:, 0:C2], in_=W2h)
                w2t = const.tile([C2, C1], F32)
                nc.sync.dma_start(out=w2t[:], in_=W2T)
                a2 = const.tile([C2, 2], F32)
                nc.sync.dma_start(out=a2[:, 0:1], in_=a2src)
                nc.sync.dma_start(out=a2[:, 1:2], in_=a2dst)
                pw2 = cpsum.tile([P, 2], F32, space="PSUM")
                nc.tensor.matmul(pw2[:], lhsT=w2t[:], rhs=a2[:], start=True,
                                 stop=True)
                nc.vector.tensor_copy(out=w2ext[:, C2:C2 + 2], in_=pw2[:])

                # broadcast bias rows to all 128 partitions (ones @ brow)
                b1sb = const.tile([1, G1C], F32)
                nc.sync.dma_start(out=b1sb[:], in_=b1row)
                b2sb = const.tile([1, G2C], F32)
                nc.sync.dma_start(out=b2sb[:], in_=b2row)
                onesb = const.tile([1, P], F32)
                nc.sync.dma_start(out=onesb[:], in_=onesrow)
                b1rep = const.tile([P, G1C], F32)
                pb1 = cpsum.tile([P, G1C], F32, space="PSUM")
                nc.tensor.matmul(pb1[:], lhsT=onesb[:], rhs=b1sb[:],
                                 start=True, stop=True)
                nc.vector.tensor_copy(out=b1rep[:], in_=pb1[:])
                b2rep = const.tile([P, G2C], F32)
                pb2 = cpsum.tile([P, G2C], F32, space="PSUM")
                nc.tensor.matmul(pb2[:], lhsT=onesb[:], rhs=b2sb[:],
                                 start=True, stop=True)
                nc.vector.tensor_copy(out=b2rep[:], in_=pb2[:])

                idsb = const.tile([P, P], F16)
                nc.sync.dma_start(out=idsb[:], in_=identh)
                adr = const.tile([P, NT], I32)
                nc.sync.dma_start(out=adr[:], in_=adrows)
                # prefetch all per-tile gather indices / validity masks
                ilosb = const.tile([P, olo[NT]], I16)
                nc.sync.dma_start(out=ilosb[:], in_=idxlo)
                ihisb = const.tile([P, ohi[NT]], I16)
                nc.sync.dma_start(out=ihisb[:], in_=idxhi)
                vmsb = const.tile([P, ovo[NT]], F16)
                nc.sync.dma_start(out=vmsb[:], in_=vmask)
                adt2sb = const.tile([P, NT], F16)
                nc.vector.memset(adt2sb[:], 0.0)

            # SBUF pools stay open across phases so later phases' tiles
            # never WAR-collide with earlier phases' addresses (lets phase-B
            # hi gathers start while phase A still writes the lo table half).
            sb_pools = tc.tile_pool(name="pa", bufs=3), \
                tc.tile_pool(name="pbig", bufs=4), \
                tc.tile_pool(name="pxx", bufs=2), \
                tc.tile_pool(name="pmed", bufs=2), \
                tc.tile_pool(name="pmsk", bufs=2), \
                tc.tile_pool(name="pdig", bufs=4), \
                tc.tile_pool(name="pdxx", bufs=2), \
                tc.tile_pool(name="pdmd", bufs=2), \
                tc.tile_pool(name="pdmk", bufs=2)
            import contextlib
            _stack = contextlib.ExitStack()
            pa, pbig, pxx, pmed, pmsk, pdig, pdxx, pdmd, pdmk = (
                _stack.enter_context(p) for p in sb_pools)
            # pre-clean gather buffers (garbage SBUF could decode as NaN f16;
            # NaN survives the 0-weight mask since 0*NaN=NaN)
            for _ in range(4):
                vgz = pbig.tile([P, SMAX * G1W], F16, tag="vg")
                nc.vector.memset(vgz[:], 0.0)
                vgz2 = pdig.tile([P, SMAX * G2W], F16, tag="vg2")
                nc.vector.memset(vgz2[:], 0.0)

            # ---- phase A: G1 rows -----------------------------------------
            with (
                tc.tile_pool(name="pap", bufs=4, space="PSUM") as pap,
            ):
                G1v = G1.rearrange("(p i) c -> p i c", i=512)
                GA = 8
                for i0 in range(0, NNT, GA):
                    gg = min(GA, NNT - i0)
                    wid = (gg - 1) * P + (P if i0 + gg < NNT else cfg.LASTN)
                    xt = pa.tile([P, GA * P], F16, tag="xt")
                    nc.sync.dma_start(out=xt[:, 0:wid],
                                      in_=xT[:, i0 * P:i0 * P + wid])
                    g1h = pa.tile([P, GA * G1W], F16, tag="g1h")
                    for g in range(gg):
                        nn = P if i0 + g < NNT - 1 else cfg.LASTN
                        ps = pap.tile([P, G1C], F32, space="PSUM", tag="ps")
                        nc.tensor.matmul(ps[:nn, :],
                                         lhsT=xt[:, g * P:g * P + nn],
                                         rhs=w1ext[:], start=True, stop=True)
                        nc.vector.tensor_tensor(
                            out=g1h[:nn, g * G1W:g * G1W + G1C],
                            in0=ps[:nn, :], in1=b1rep[:nn, :], op=OP.add)
                    # 128 contiguous gg*512B descriptors per group
                    nc.scalar.dma_start(
                        out=G1v[:, i0:i0 + gg, :],
                        in_=g1h[:, 0:gg * G1W]
                            .rearrange("p (g c) -> p g c", g=gg))

            # scatter an allgathered node-order block into the permuted
            # G2 table: per core the row range is split [head | aligned |
            # tail] so each piece is a dense 2-3D access pattern
            G2v = G2.rearrange("(p i) c -> p i c", i=512)

            def _relayout(j):
                w = ROW0[j + 1] - ROW0[j]
                for cc in range(cfg.NC):
                    a0 = cc * NPC + ROW0[j]        # global node range start
                    src_ap = G2cfs[j][cc * w:(cc + 1) * w, :]
                    pieces = []
                    h = (-a0) % P
                    if h:
                        pieces.append((0, min(h, w)))
                    m0 = min(h, w)
                    m1 = m0 + ((w - m0) // P) * P
                    if m1 > m0:
                        pieces.append((m0, m1))
                    if w > m1:
                        pieces.append((m1, w))
                    for (o0, o1) in pieces:
                        n0 = a0 + o0
                        cnt = o1 - o0
                        if cnt >= P:
                            nb = cnt // P
                            nc.sync.dma_start(
                                out=G2v[:, n0 // P:n0 // P + nb, 0:G2C],
                                in_=src_ap[o0:o1, :]
                                    .rearrange("(a b) c -> b a c", b=P))
                        else:
                            p0 = n0 % P
                            nc.sync.dma_start(
                                out=G2v[p0:p0 + cnt, n0 // P, 0:G2C],
                                in_=src_ap[o0:o1, :])

            # ---- phase B: layer-1 edges + layer-2 row build ---------------
            with (
                tc.tile_pool(name="pbp", bufs=2, space="PSUM") as pbp,
                tc.tile_pool(name="pbpa", bufs=2, space="PSUM") as pbpa,
                tc.tile_pool(name="pcp", bufs=1, space="PSUM") as pcp,
                tc.tile_pool(name="pcpt", bufs=1, space="PSUM") as pcpt,
            ):
                for t in range(NT):
                    ndst = P if t < NT - 1 else cfg.LAST
                    CLt, CHt, St = CLs[t], CHs[t], Ss[t]
                    vg = pbig.tile([P, SMAX * G1W], F16, tag="vg")
                    vg3 = vg[:].rearrange("p (k c) -> p k c", c=G1W)
                    if CHt:
                        nc.gpsimd.dma_gather(
                            out_ap=vg3[:, CLt:St, :], in_ap=G1[HALF:NROWS, :],
                            idxs_ap=ihisb[:, ohi[t]:ohi[t] + CHt * 8],
                            num_idxs=CHt * P, num_idxs_reg=CHt * P,
                            elem_size=G1W, single_packet=False)
                    if CLt:
                        nc.gpsimd.dma_gather(
                            out_ap=vg3[:, 0:CLt, :],
                            in_ap=G1[0:HALF, :],
                            idxs_ap=ilosb[:, olo[t]:olo[t] + CLt * 8],
                            num_idxs=CLt * P, num_idxs_reg=CLt * P,
                            elem_size=G1W, single_packet=False)
                    # alpha_dst values of this tile's dsts, per partition
                    adt = pmed.tile([P, H], F16, tag="adt")
                    nc.gpsimd.indirect_dma_start(
                        out=adt[:], out_offset=None, in_=G1,
                        in_offset=IndirectOffsetOnAxis(ap=adr[:, t:t + 1],
                                                       axis=0),
                        element_offset=C1 + H)
                    msk = pmsk.tile([P, SMAX * P], F16, tag="msk")
                    nc.sync.dma_start(
                        out=msk[:, 0:St * P],
                        in_=ovmask[:, omo[t]:omo[t] + St * P])
                    mskT = pmsk.tile([P, SMAX * P], F16, tag="mskT")
                    nc.scalar.dma_start(
                        out=mskT[:, 0:St * P],
                        in_=ovmaskT[:, omo[t]:omo[t] + St * P])

                    # per-slot alpha_dst for every chunk: maskT @ adt
                    adp = pbpa.tile([P, SMAX * H], F32, space="PSUM",
                                    tag="adp")
                    for k in range(St):
                        nc.tensor.matmul(
                            adp[:, k * H:(k + 1) * H],
                            lhsT=mskT[:, k * P:(k + 1) * P],
                            rhs=adt[:], start=True, stop=True)
                    alp = pmed.tile([P, SMAX * H], F16, tag="alp")
                    alp3 = alp[:].rearrange("p (k h) -> p k h", h=H)
                    nc.vector.tensor_tensor(
                        out=alp3[:, 0:St, :],
                        in0=vg3[:, 0:St, C1:C1 + H],
                        in1=adp[:].rearrange("p (k h) -> p k h", h=H)
                            [:, 0:St, :],
                        op=OP.add)
                    # e = exp(lrelu(alpha)) * vmask
                    asc = pmed.tile([P, SMAX * H], F16, tag="asc")
                    nc.vector.tensor_scalar(out=asc[:, 0:St * H],
                                            in0=alp[:, 0:St * H],
                                            scalar1=cfg.NEG, scalar2=None,
                                            op0=OP.mult)
                    lrl = pmed.tile([P, SMAX * H], F16, tag="lrl")
                    nc.vector.tensor_tensor(out=lrl[:, 0:St * H],
                                            in0=alp[:, 0:St * H],
                                            in1=asc[:, 0:St * H], op=OP.max)
                    ee = pmed.tile([P, SMAX * H], F32, tag="ee")
                    nc.scalar.activation(out=ee[:, 0:St * H],
                                         in_=lrl[:, 0:St * H], func=AF.Exp)
                    eeh = pmed.tile([P, SMAX * H], F16, tag="eeh")
                    nc.vector.tensor_tensor(
                        out=eeh[:, 0:St * H].rearrange("p (k h) -> p k h",
                                                       h=H),
                        in0=ee[:, 0:St * H].rearrange("p (k h) -> p k h",
                                                      h=H),
                        in1=vmsb[:, ovo[t]:ovo[t] + St]
                            .rearrange("p (k o) -> p k o", o=1)
                            .to_broadcast([P, St, H]),
                        op=OP.mult)
                    eeh3 = eeh[:].rearrange("p (k h) -> p k h", h=H)
                    # rhs = [e*h | e]
                    xx = pxx.tile([P, SMAX * (C1 + H)], F16, tag="xx")
                    xx3 = xx[:].rearrange("p (k c) -> p k c", c=C1 + H)
                    nc.vector.tensor_copy(out=xx3[:, 0:St, C1:C1 + H],
                                          in_=eeh3[:, 0:St, :])
                    nc.vector.tensor_tensor(
                        out=xx3[:, 0:St, 0:C1].rearrange(
                            "p k (h c) -> p k h c", c=HID),
                        in0=vg3[:, 0:St, 0:C1].rearrange(
                            "p k (h c) -> p k h c", c=HID),
                        in1=eeh[:, 0:St * H].rearrange(
                            "p (k h o) -> p k h o", h=H, o=1)
                            .to_broadcast([P, St, H, HID]),
                        op=OP.mult)
                    ps = pbp.tile([P, C1 + H], F32, space="PSUM", tag="ps")
                    for k in range(St):
                        nc.tensor.matmul(ps[:], lhsT=msk[:, k * P:(k + 1) * P],
                                         rhs=xx3[:, k, :],
                                         start=(k == 0), stop=(k == St - 1))
                    rec = pmed.tile([P, H], F32, tag="rec")
                    nc.vector.reciprocal(out=rec[:ndst, :],
                                         in_=ps[:ndst, C1:C1 + H])
                    o1 = pmed.tile([P, C1], F16, tag="o1")
                    if ndst < P:
                        nc.vector.memset(o1[:], 0.0)
                    for h in range(H):
                        nc.vector.tensor_scalar(
                            out=o1[:ndst, h * HID:(h + 1) * HID],
                            in0=ps[:ndst, h * HID:(h + 1) * HID],
                            scalar1=rec[:ndst, h:h + 1], scalar2=0.0,
                            op0=OP.mult, op1=OP.max)
                    # layer-2 row build: transpose + project
                    tp = pcpt.tile([P, P], F16, space="PSUM", tag="tp")
                    nc.tensor.transpose(out=tp[:], in_=o1[:],
                                        identity=idsb[:])
                    o1t = pmed.tile([P, P], F16, tag="o1t")
                    nc.vector.tensor_copy(out=o1t[:], in_=tp[:])
                    hp = pcp.tile([P, G2C], F32, space="PSUM", tag="hp")
                    nc.tensor.matmul(hp[:], lhsT=o1t[:], rhs=w2ext[:],
                                     start=True, stop=True)
                    g2h = pmed.tile([P, G2C], F16, tag="g2h")
                    nc.vector.tensor_tensor(out=g2h[:ndst, :],
                                            in0=hp[:ndst, :],
                                            in1=b2rep[:ndst, :], op=OP.add)
                    nc.vector.tensor_copy(out=adt2sb[:ndst, t:t + 1],
                                          in_=g2h[:ndst, C2 + 1:C2 + 2])
                    nc.sync.dma_start(out=G2c[t * P:t * P + ndst, :],
                                      in_=g2h[:ndst, :])
                    if t + 1 in TSPLIT:
                        # partial table replication overlaps the rest of B
                        j = TSPLIT.index(t + 1)
                        nc.gpsimd.collective_compute(
                            "AllGather", OP.bypass,
                            replica_groups=[list(range(cfg.NC))],
                            ins=[G2c[ROW0[j]:ROW0[j + 1], :]],
                            outs=[G2cfs[j]])
                        _relayout(j)

            nc.gpsimd.collective_compute(
                "AllGather", OP.bypass,
                replica_groups=[list(range(cfg.NC))],
                ins=[G2c[ROW0[2]:ROW0[3], :]], outs=[G2cfs[2]])
            _relayout(2)

            # ---- phase D: layer-2 edge aggregation (1 head) ---------------
            with (
                tc.tile_pool(name="pdp", bufs=2, space="PSUM") as pdp,
                tc.tile_pool(name="pdpa", bufs=2, space="PSUM") as pdpa,
            ):
                for t in range(NT):
                    ndst = P if t < NT - 1 else cfg.LAST
                    CLt, CHt, St = CLs[t], CHs[t], Ss[t]
                    vg = pdig.tile([P, SMAX * G2W], F16, tag="vg2")
                    vg3 = vg[:].rearrange("p (k c) -> p k c", c=G2W)
                    if CLt:
                        nc.gpsimd.dma_gather(
                            out_ap=vg3[:, 0:CLt, :],
                            in_ap=G2[0:HALF, :],
                            idxs_ap=ilosb[:, olo[t]:olo[t] + CLt * 8],
                            num_idxs=CLt * P, num_idxs_reg=CLt * P,
                            elem_size=G2W, single_packet=False)
                    if CHt:
                        nc.gpsimd.dma_gather(
                            out_ap=vg3[:, CLt:St, :], in_ap=G2[HALF:NROWS, :],
                            idxs_ap=ihisb[:, ohi[t]:ohi[t] + CHt * 8],
                            num_idxs=CHt * P, num_idxs_reg=CHt * P,
                            elem_size=G2W, single_packet=False)
                    adt = adt2sb[:, t:t + 1]
                    msk = pdmk.tile([P, SMAX * P], F16, tag="msk2")
                    nc.sync.dma_start(
                        out=msk[:, 0:St * P],
                        in_=ovmask[:, omo[t]:omo[t] + St * P])
                    mskT = pdmk.tile([P, SMAX * P], F16, tag="mskT2")
                    nc.scalar.dma_start(
                        out=mskT[:, 0:St * P],
                        in_=ovmaskT[:, omo[t]:omo[t] + St * P])

                    adp = pdpa.tile([P, SMAX], F32, space="PSUM", tag="adp2")
                    for k in range(St):
                        nc.tensor.matmul(
                            adp[:, k:k + 1],
                            lhsT=mskT[:, k * P:(k + 1) * P],
                            rhs=adt, start=True, stop=True)
                    alp = pdmd.tile([P, SMAX], F16, tag="alp2")
                    alp3 = alp[:].rearrange("p (k o) -> p k o", o=1)
                    nc.vector.tensor_tensor(
                        out=alp3[:, 0:St, :],
                        in0=vg3[:, 0:St, C2:C2 + 1],
                        in1=adp[:].rearrange("p (k o) -> p k o", o=1)
                            [:, 0:St, :],
                        op=OP.add)
                    asc = pdmd.tile([P, SMAX], F16, tag="asc2")
                    nc.vector.tensor_scalar(out=asc[:, 0:St],
                                            in0=alp[:, 0:St],
                                            scalar1=cfg.NEG, scalar2=None,
                                            op0=OP.mult)
                    lrl = pdmd.tile([P, SMAX], F16, tag="lrl2")
                    nc.vector.tensor_tensor(out=lrl[:, 0:St],
                                            in0=alp[:, 0:St],
                                            in1=asc[:, 0:St], op=OP.max)
                    ee = pdmd.tile([P, SMAX], F32, tag="ee2")
                    nc.scalar.activation(out=ee[:, 0:St], in_=lrl[:, 0:St],
                                         func=AF.Exp)
                    eeh = pdmd.tile([P, SMAX], F16, tag="eeh2")
                    nc.vector.tensor_tensor(out=eeh[:, 0:St],
                                            in0=ee[:, 0:St],
                                            in1=vmsb[:, ovo[t]:ovo[t] + St],
                                            op=OP.mult)
                    xx = pdxx.tile([P, SMAX * (C2 + 1)], F16, tag="xx2")
                    xx3 = xx[:].rearrange("p (k c) -> p k c", c=C2 + 1)
                    nc.vector.tensor_copy(
                        out=xx3[:, 0:St, C2:C2 + 1],
                        in_=eeh[:, 0:St].rearrange("p (k o) -> p k o", o=1))
                    nc.vector.tensor_tensor(
                        out=xx3[:, 0:St, 0:C2],
                        in0=vg3[:, 0:St, 0:C2],
                        in1=eeh[:, 0:St].rearrange("p (k o) -> p k o", o=1)
                            .to_broadcast([P, St, C2]),
                        op=OP.mult)
                    ps = pdp.tile([P, C2 + 1], F32, space="PSUM", tag="ps2")
                    for k in range(St):
                        nc.tensor.matmul(ps[:], lhsT=msk[:, k * P:(k + 1) * P],
                                         rhs=xx3[:, k, :],
                                         start=(k == 0), stop=(k == St - 1))
                    rec = pdmd.tile([P, 1], F32, tag="rec2")
                    nc.vector.reciprocal(out=rec[:ndst, :],
                                         in_=ps[:ndst, C2:C2 + 1])
                    o2 = pdmd.tile([P, C2], F32, tag="o2")
                    nc.vector.tensor_scalar(out=o2[:ndst, :],
                                            in0=ps[:ndst, 0:C2],
                                            scalar1=rec[:ndst, :],
                                            scalar2=None, op0=OP.mult)
                    nc.sync.dma_start(out=PRE[t * P:t * P + ndst, :],
                                      in_=o2[:ndst, :])

            _stack.close()

            # ---- phase E: one sigmoid sweep -------------------------------
            with tc.tile_pool(name="pe", bufs=2) as pe:
                FW = NT * C2
                pre_f = PRE.rearrange("(a b) c -> a (b c)", a=P)
                out_f = out.rearrange("(a b) c -> a (b c)", a=P)
                pei = pe.tile([P, FW], F32, tag="pei")
                nc.sync.dma_start(out=pei[:], in_=pre_f)
                peo = pe.tile([P, FW], F32, tag="peo")
                nc.scalar.activation(out=peo[:], in_=pei[:], func=AF.Sigmoid)
                nc.sync.dma_start(out=out_f, in_=peo[:])

    nc.compile()
    return nc


_CACHE: dict = {}


def _get_module(cfg, plan):
    key = (cfg.N, cfg.E, plan.CLs, plan.CHs)
    if key not in _CACHE:
        _CACHE[key] = _build(cfg, plan)
    return _CACHE[key]


def _run(cfg, inputs, trace=False):
    plan, common, per_core = _prep_host(
        cfg, inputs["x"], inputs["edge_index"], inputs["W1"],
        inputs["a_src1"], inputs["a_dst1"], inputs["b1"], inputs["W2"],
        inputs["a_src2"], inputs["a_dst2"], inputs["b2"])
    nc = _get_module(cfg, plan)
    in_maps = [dict(common, **pc) for pc in per_core]
    res = run_bass_kernel_spmd(nc, in_maps, core_ids=list(range(cfg.NC)),
                               trace=trace)
    shards = [np.asarray(res.results[c]["out"])[:cfg.NPC]
              for c in range(cfg.NC)]
    full = np.concatenate(shards, axis=0).astype(np.float32)
    return (full, res) if trace else full


def kernel(**inputs) -> np.ndarray:
    cfg = GATConfig(n=50000, e=800000)
    return _run(cfg, inputs)


# revision 16
# speedup vs baseline: 1.1368x; 1.0288x over previous
"""GAT 2-layer propagation kernel for Trainium2, 8 NeuronCores (SPMD).

Strategy (edge-parallel, dst-node-range sharded across 8 cores):
  - Core c owns dst nodes [c*6250, (c+1)*6250); edges (with self-loops) go to
    the core owning their dst, so each core emits its contiguous output rows
    and no reduction collective is needed.
  - Per layer a DRAM gather table holds one fp16 row per node:
      G1[n] = [h1 x128 | as1 x4 | ad1 x4 | pad]   (512 B rows)
      G2[n] = [h2 x32  | as2    | ad2    | pad]   (256 B rows)
    (h carries the layer bias folded in: softmax weights sum to 1, so adding
    b to every value row adds b to the output.)
  - Per 128-dst tile, edges are packed DENSELY into chunks of 128 slots
    (partition dim), lo-src chunks then hi-src chunks (dma_gather indices
    are int16, so the node table is addressed in two halves).  Dense packing
    needs ~19 chunks/tile vs ~21 for the aligned layout, directly shrinking
    GpSimd descriptor generation (the serial bottleneck, ~8ns/row).  Unused
    tail slots gather row 0; the validity mask zeroes their weight.
  - Each chunk k has a host-precomputed [slot, dst] 0/1 selection mask (and
    its transpose).  maskT @ adt recovers per-slot alpha_dst; mask is the
    stationary operand of the per-chunk segment-sum matmul, which
    accumulates [sum e*h | sum e] in fp32 PSUM across chunks; the softmax
    division happens once per dst at the end.
  - e = exp(leakyrelu(as+ad)) needs no max-subtraction (|alpha| <= ~6 here);
    exp runs in fp32 so stale-slot garbage stays finite, and the validity
    mask zeroes it before the fp16 value multiply.
  - Layer-2 table rows are built inline as each layer-1 output tile
    finishes; an AllGather + relayout replicates the table. The final
    sigmoid runs as one deferred sweep so ACT's activation table stays on
    Exp during edge phases.
"""

import numpy as np

import concourse.bacc as bacc
import concourse.tile as tile
from concourse import mybir
from concourse.bass import IndirectOffsetOnAxis
from concourse.bass_utils import run_bass_kernel_spmd

F32 = mybir.dt.float32
F16 = mybir.dt.float16
I32 = mybir.dt.int32
I16 = mybir.dt.int16
AF = mybir.ActivationFunctionType
OP = mybir.AluOpType

P = 128
HALF = 32768            # int16-addressable rows per dma_gather call


class GATConfig:
    def __init__(self, n, e, in_dim=128, hid=32, heads=4, out_dim=32,
                 neg_slope=0.2, n_cores=8):
        assert in_dim == P and heads * hid == P
        self.N, self.E = n, e
        self.HID = hid
        self.H = heads
        self.OUT = out_dim
        self.NEG = neg_slope
        self.NC = n_cores
        assert n % n_cores == 0
        self.NPC = n // n_cores
        self.NT = (self.NPC + P - 1) // P
        self.LAST = self.NPC - (self.NT - 1) * P
        self.C1 = heads * hid                 # 128
        self.G1W = 256                        # fp16 els/row: h|as|ad|pad
        self.G2W = 128
        self.NNT = (n + P - 1) // P
        self.LASTN = n - (self.NNT - 1) * P


class EdgePlan:
    """Dense chunk structure: per-tile chunk counts (max over cores)."""


def _prep_host(cfg, x, edge_index, W1, a_src1, a_dst1, b1, W2, a_src2,
               a_dst2, b2):
    N, H, HID = cfg.N, cfg.H, cfg.HID
    NPC, NT, NC = cfg.NPC, cfg.NT, cfg.NC

    src = np.concatenate([np.asarray(edge_index[0], dtype=np.int64),
                          np.arange(N, dtype=np.int64)])
    dst = np.concatenate([np.asarray(edge_index[1], dtype=np.int64),
                          np.arange(N, dtype=np.int64)])
    order = np.argsort(dst, kind="stable")
    src, dst = src[order], dst[order]

    core_of = dst // NPC
    tile_of = (dst % NPC) // P
    part_of = (dst % NPC) % P
    # permuted table row: node n lives at (n%128)*512 + n//128, so the
    # int16 lo/hi table halves are the p<64 / p>=64 partition slices
    psrc = (src % P) * 512 + src // P
    is_hi = psrc >= HALF

    nlo = np.zeros((NC, NT), np.int64)
    nhi = np.zeros((NC, NT), np.int64)
    np.add.at(nlo, (core_of[~is_hi], tile_of[~is_hi]), 1)
    np.add.at(nhi, (core_of[is_hi], tile_of[is_hi]), 1)

    CLs = np.ceil(nlo.max(axis=0) / P).astype(int)     # per-tile, all cores
    CHs = np.ceil(nhi.max(axis=0) / P).astype(int)
    Ss = CLs + CHs
    plan = EdgePlan()
    plan.CLs, plan.CHs = tuple(int(v) for v in CLs), tuple(int(v) for v in CHs)
    plan.Ss = tuple(int(v) for v in Ss)
    plan.SMAX = int(Ss.max())
    olo = np.concatenate([[0], np.cumsum(CLs * 8)])    # idx col offsets
    ohi = np.concatenate([[0], np.cumsum(CHs * 8)])
    ovo = np.concatenate([[0], np.cumsum(Ss)])         # vmask col offsets
    omo = np.concatenate([[0], np.cumsum(Ss * P)])     # mask col offsets
    plan.olo, plan.ohi = tuple(olo.tolist()), tuple(ohi.tolist())
    plan.ovo, plan.omo = tuple(ovo.tolist()), tuple(omo.tolist())
    TLO, THI, TVM, TMK = olo[-1], ohi[-1], ovo[-1], omo[-1]

    def pack16(vals):
        # vals: [n_chunks*P] int16 in slot order j=k*128+p; idx j lives at
        # [j%16, j//16], replicated across the 8 stripes of 16 partitions.
        a = vals.reshape(-1, 16).T.astype(np.int16)   # [16, n/16]
        return np.tile(a, (8, 1))

    per_core = []
    for c in range(NC):
        m = core_of == c
        s_c = psrc[m].astype(np.int64)
        t_c = tile_of[m]
        p_c = part_of[m]
        ilo = np.zeros((P, TLO), np.int16)
        ihi = np.zeros((P, THI), np.int16)
        vmk = np.zeros((P, TVM), np.float16)
        ovm = np.zeros((P, TMK), np.float16)
        ovmT = np.zeros((P, TMK), np.float16)
        for t in range(NT):
            CLt, CHt = CLs[t], CHs[t]
            mt = t_c == t
            s_t, p_t = s_c[mt], p_c[mt]
            lo_t = s_t < HALF
            for half in (0, 1):
                if half == 0:
                    ss, pp = s_t[lo_t], p_t[lo_t]
                    nch, koff = CLt, 0
                else:
                    ss, pp = s_t[~lo_t] - HALF, p_t[~lo_t]
                    nch, koff = CHt, CLt
                if nch == 0:
                    continue
                iv = np.zeros(nch * P, np.int16)
                ne = len(ss)
                iv[:ne] = ss
                jj = np.arange(ne)
                kk, sl = jj // P, jj % P
                vmk[sl, ovo[t] + koff + kk] = 1.0
                ovm[sl, omo[t] + (koff + kk) * P + pp] = 1.0
                ovmT[pp, omo[t] + (koff + kk) * P + sl] = 1.0
                pk = pack16(iv)
                if half == 0:
                    ilo[:, olo[t]:olo[t] + nch * 8] = pk
                else:
                    ihi[:, ohi[t]:ohi[t] + nch * 8] = pk

        nrow = (c * NPC + np.arange(NT)[None, :] * P
                + np.arange(P)[:, None])
        np.clip(nrow, 0, N - 1, out=nrow)
        adrows = ((nrow % P) * 512 + nrow // P).astype(np.int32)
        per_core.append({
            "idxlo": np.ascontiguousarray(ilo),
            "idxhi": np.ascontiguousarray(ihi),
            "vmask": np.ascontiguousarray(vmk),
            "ovmask": np.ascontiguousarray(ovm),
            "ovmaskT": np.ascontiguousarray(ovmT),
            "adrows": np.ascontiguousarray(adrows),
        })

    # block-diagonal attention projectors: as1 = h1 @ asrc_blk
    asrc_blk = np.zeros((cfg.C1, H), np.float32)
    adst_blk = np.zeros((cfg.C1, H), np.float32)
    for h in range(H):
        asrc_blk[h * HID:(h + 1) * HID, h] = a_src1[h]
        adst_blk[h * HID:(h + 1) * HID, h] = a_dst1[h]

    b1row = np.zeros((1, cfg.C1 + 2 * H), np.float32)
    b1row[0, :cfg.C1] = b1
    b2row = np.zeros((1, cfg.OUT + 2), np.float32)
    b2row[0, :cfg.OUT] = b2

    common = {
        "xT": np.ascontiguousarray(np.asarray(x, np.float16).T),
        "W1h": np.ascontiguousarray(np.asarray(W1, np.float16)),
        "W1T": np.ascontiguousarray(np.asarray(W1, np.float32).T),
        "asrcblk": asrc_blk, "adstblk": adst_blk, "b1row": b1row,
        "W2h": np.ascontiguousarray(np.asarray(W2, np.float16)),
        "W2T": np.ascontiguousarray(np.asarray(W2, np.float32).T),
        "a2src": np.ascontiguousarray(
            np.asarray(a_src2, np.float32).reshape(-1, 1)),
        "a2dst": np.ascontiguousarray(
            np.asarray(a_dst2, np.float32).reshape(-1, 1)),
        "b2row": b2row,
        "identh": np.eye(P, dtype=np.float16),
        "onesrow": np.ones((1, P), np.float32),
    }
    return plan, common, per_core


def _build(cfg, plan):
    N, H, HID, C1 = cfg.N, cfg.H, cfg.HID, cfg.C1
    NT, NPC, NNT = cfg.NT, cfg.NPC, cfg.NNT
    C2 = cfg.OUT
    CLs, CHs, Ss, SMAX = plan.CLs, plan.CHs, plan.Ss, plan.SMAX
    olo, ohi, ovo, omo = plan.olo, plan.ohi, plan.ovo, plan.omo
    G1W, G2W = cfg.G1W, cfg.G2W
    G1C = C1 + 2 * H                       # 136 used cols in G1 rows
    G2C = C2 + 2                           # 34 used cols in G2 rows

    nc = bacc.Bacc("TRN2", target_bir_lowering=False, debug=False,
                   num_devices=cfg.NC)

    def din(name, shape, dt=F32):
        return nc.dram_tensor(name, shape, dt, kind="ExternalInput").ap()

    xT = din("xT", [P, N], F16)
    W1h = din("W1h", [P, C1], F16)
    W1T = din("W1T", [C1, P])
    asrcblk = din("asrcblk", [C1, H])
    adstblk = din("adstblk", [C1, H])
    b1row = din("b1row", [1, G1C])
    W2h = din("W2h", [C1, C2], F16)
    W2T = din("W2T", [C2, C1])
    a2src = din("a2src", [C2, 1])
    a2dst = din("a2dst", [C2, 1])
    b2row = din("b2row", [1, G2C])
    identh = din("identh", [P, P], F16)
    onesrow = din("onesrow", [1, P])
    idxlo = din("idxlo", [P, olo[NT]], I16)
    idxhi = din("idxhi", [P, ohi[NT]], I16)
    vmask = din("vmask", [P, ovo[NT]], F16)
    ovmask = din("ovmask", [P, omo[NT]], F16)
    ovmaskT = din("ovmaskT", [P, omo[NT]], F16)
    adrows = din("adrows", [P, NT], I32)

    out = nc.dram_tensor("out", [NT * P, C2], F32, kind="ExternalOutput").ap()

    NROWS = P * 512                        # permuted node-table rows
    G1 = nc.dram_tensor("G1", [NROWS, G1W], F16).ap()
    G2 = nc.dram_tensor("G2", [NROWS, G2W], F16).ap()
    G2c = nc.dram_tensor("G2c", [NPC, G2C], F16).ap()
    TSPLIT = (20, 36, 47)                  # allgather after these tiles
    NCH = len(TSPLIT) + 1
    ROW0 = (0,) + tuple(tt * P for tt in TSPLIT) + (NPC,)
    G2cfs = [nc.dram_tensor(f"G2cf{j}",
                            [cfg.NC * (ROW0[j + 1] - ROW0[j]), G2C], F16,
                            addr_space="Shared").ap() for j in range(NCH)]

    with tile.TileContext(nc) as tc:
        with tc.tile_pool(name="const", bufs=1) as const:
            # ---- constants / fused weight tables --------------------------
            with tc.tile_pool(name="cpsum", bufs=1, space="PSUM") as cpsum:
                w1ext = const.tile([P, G1C], F16)   # [W1 | W1@Asrc | W1@Adst]
                nc.sync.dma_start(out=w1ext[:, 0:C1], in_=W1h)
                w1t = const.tile([P, P], F32)
                nc.sync.dma_start(out=w1t[:], in_=W1T)
                ablk = const.tile([P, 2 * H], F32)
                nc.sync.dma_start(out=ablk[:, 0:H], in_=asrcblk)
                nc.sync.dma_start(out=ablk[:, H:2 * H], in_=adstblk)
                pw = cpsum.tile([P, 2 * H], F32, space="PSUM")
                nc.tensor.matmul(pw[:], lhsT=w1t[:], rhs=ablk[:], start=True,
                                 stop=True)
                nc.vector.tensor_copy(out=w1ext[:, C1:C1 + 2 * H], in_=pw[:])

                w2ext = const.tile([P, G2C], F16)   # [W2 | W2@a2s | W2@a2d]
                nc.sync.dma_start(out=w2ext[:, 0:C2], in_=W2h)
                w2t = const.tile([C2, C1], F32)
                nc.sync.dma_start(out=w2t[:], in_=W2T)
                a2 = const.tile([C2, 2], F32)
                nc.sync.dma_start(out=a2[:, 0:1], in_=a2src)
                nc.sync.dma_start(out=a2[:, 1:2], in_=a2dst)
                pw2 = cpsum.tile([P, 2], F32, space="PSUM")
                nc.tensor.matmul(pw2[:], lhsT=w2t[:], rhs=a2[:], start=True,
                                 stop=True)
                nc.vector.tensor_copy(out=w2ext[:, C2:C2 + 2], in_=pw2[:])

                # broadcast bias rows to all 128 partitions (ones @ brow)
                b1sb = const.tile([1, G1C], F32)
                nc.sync.dma_start(out=b1sb[:], in_=b1row)
                b2sb = const.tile([1, G2C], F32)
                nc.sync.dma_start(out=b2sb[:], in_=b2row)
                onesb = const.tile([1, P], F32)
                nc.sync.dma_start(out=onesb[:], in_=onesrow)
                b1rep = const.tile([P, G1C], F32)
                pb1 = cpsum.tile([P, G1C], F32, space="PSUM")
                nc.tensor.matmul(pb1[:], lhsT=onesb[:], rhs=b1sb[:],
                                 start=True, stop=True)
                nc.vector.tensor_copy(out=b1rep[:], in_=pb1[:])
                b2rep = const.tile([P, G2C], F32)
                pb2 = cpsum.tile([P, G2C], F32, space="PSUM")
                nc.tensor.matmul(pb2[:], lhsT=onesb[:], rhs=b2sb[:],
                                 start=True, stop=True)
                nc.vector.tensor_copy(out=b2rep[:], in_=pb2[:])

                idsb = const.tile([P, P], F16)
                nc.sync.dma_start(out=idsb[:], in_=identh)
                adr = const.tile([P, NT], I32)
                nc.sync.dma_start(out=adr[:], in_=adrows)
                # prefetch all per-tile gather indices / validity masks
                ilosb = const.tile([P, olo[NT]], I16)
                nc.sync.dma_start(out=ilosb[:], in_=idxlo)
                ihisb = const.tile([P, ohi[NT]], I16)
                nc.sync.dma_start(out=ihisb[:], in_=idxhi)
                vmsb = const.tile([P, ovo[NT]], F16)
                nc.sync.dma_start(out=vmsb[:], in_=vmask)
                adt2sb = const.tile([P, NT], F16)
                nc.vector.memset(adt2sb[:], 0.0)

            # SBUF pools stay open across phases so later phases' tiles
            # never WAR-collide with earlier phases' addresses (lets phase-B
            # hi gathers start while phase A still writes the lo table half).
            sb_pools = tc.tile_pool(name="pa", bufs=3), \
                tc.tile_pool(name="pbig", bufs=4), \
                tc.tile_pool(name="pxx", bufs=2), \
                tc.tile_pool(name="pmed", bufs=2), \
                tc.tile_pool(name="pmsk", bufs=2), \
                tc.tile_pool(name="pdig", bufs=4), \
                tc.tile_pool(name="pdxx", bufs=2), \
                tc.tile_pool(name="pdmd", bufs=2), \
                tc.tile_pool(name="pdmk", bufs=2)
            import contextlib
            _stack = contextlib.ExitStack()
            pa, pbig, pxx, pmed, pmsk, pdig, pdxx, pdmd, pdmk = (
                _stack.enter_context(p) for p in sb_pools)
            # pre-clean gather buffers (garbage SBUF could decode as NaN f16;
            # NaN survives the 0-weight mask since 0*NaN=NaN)
            for _ in range(4):
                vgz = pbig.tile([P, SMAX * G1W], F16, tag="vg")
                nc.vector.memset(vgz[:], 0.0)
                vgz2 = pdig.tile([P, SMAX * G2W], F16, tag="vg2")
                nc.vector.memset(vgz2[:], 0.0)

            # ---- phase A: G1 rows -----------------------------------------
            with (
                tc.tile_pool(name="pap", bufs=4, space="PSUM") as pap,
            ):
                G1v = G1.rearrange("(p i) c -> p i c", i=512)
                GA = 8
                for i0 in range(0, NNT, GA):
                    gg = min(GA, NNT - i0)
                    wid = (gg - 1) * P + (P if i0 + gg < NNT else cfg.LASTN)
                    xt = pa.tile([P, GA * P], F16, tag="xt")
                    nc.sync.dma_start(out=xt[:, 0:wid],
                                      in_=xT[:, i0 * P:i0 * P + wid])
                    g1h = pa.tile([P, GA * G1W], F16, tag="g1h")
                    for g0 in range(0, gg, 2):
                        gp = min(2, gg - g0)
                        ps = pap.tile([P, 2 * G1C], F32, space="PSUM",
                                      tag="ps")
                        for g in range(g0, g0 + gp):
                            nn = P if i0 + g < NNT - 1 else cfg.LASTN
                            nc.tensor.matmul(
                                ps[:nn, (g - g0) * G1C:(g - g0 + 1) * G1C],
                                lhsT=xt[:, g * P:g * P + nn],
                                rhs=w1ext[:], start=True, stop=True)
                        nc.vector.tensor_tensor(
                            out=g1h[:, g0 * G1W:(g0 + gp) * G1W]
                                .rearrange("p (g c) -> p g c", g=gp)
                                [:, :, 0:G1C],
                            in0=ps[:].rearrange("p (g c) -> p g c", g=2)
                                [:, 0:gp, :],
                            in1=b1rep[:].rearrange("p (o c) -> p o c", o=1)
                                .to_broadcast([P, gp, G1C]),
                            op=OP.add)
                    # 128 contiguous gg*512B descriptors per group
                    nc.scalar.dma_start(
                        out=G1v[:, i0:i0 + gg, :],
                        in_=g1h[:, 0:gg * G1W]
                            .rearrange("p (g c) -> p g c", g=gg))

            # scatter an allgathered node-order block into the permuted
            # G2 table: per core the row range is split [head | aligned |
            # tail] so each piece is a dense 2-3D access pattern
            G2v = G2.rearrange("(p i) c -> p i c", i=512)

            def _relayout(j):
                w = ROW0[j + 1] - ROW0[j]
                for cc in range(cfg.NC):
                    a0 = cc * NPC + ROW0[j]        # global node range start
                    src_ap = G2cfs[j][cc * w:(cc + 1) * w, :]
                    pieces = []
                    h = (-a0) % P
                    if h:
                        pieces.append((0, min(h, w)))
                    m0 = min(h, w)
                    m1 = m0 + ((w - m0) // P) * P
                    if m1 > m0:
                        pieces.append((m0, m1))
                    if w > m1:
                        pieces.append((m1, w))
                    for (o0, o1) in pieces:
                        n0 = a0 + o0
                        cnt = o1 - o0
                        if cnt >= P:
                            nb = cnt // P
                            nc.sync.dma_start(
                                out=G2v[:, n0 // P:n0 // P + nb, 0:G2C],
                                in_=src_ap[o0:o1, :]
                                    .rearrange("(a b) c -> b a c", b=P))
                        else:
                            p0 = n0 % P
                            nc.sync.dma_start(
                                out=G2v[p0:p0 + cnt, n0 // P, 0:G2C],
                                in_=src_ap[o0:o1, :])

            # ---- phase B: layer-1 edges + layer-2 row build ---------------
            with (
                tc.tile_pool(name="pbp", bufs=2, space="PSUM") as pbp,
                tc.tile_pool(name="pbpa", bufs=2, space="PSUM") as pbpa,
                tc.tile_pool(name="pcp", bufs=1, space="PSUM") as pcp,
                tc.tile_pool(name="pcpt", bufs=1, space="PSUM") as pcpt,
            ):
                for t in range(NT):
                    ndst = P if t < NT - 1 else cfg.LAST
                    CLt, CHt, St = CLs[t], CHs[t], Ss[t]
                    vg = pbig.tile([P, SMAX * G1W], F16, tag="vg")
                    vg3 = vg[:].rearrange("p (k c) -> p k c", c=G1W)
                    if CHt:
                        nc.gpsimd.dma_gather(
                            out_ap=vg3[:, CLt:St, :], in_ap=G1[HALF:NROWS, :],
                            idxs_ap=ihisb[:, ohi[t]:ohi[t] + CHt * 8],
                            num_idxs=CHt * P, num_idxs_reg=CHt * P,
                            elem_size=G1W, single_packet=False)
                    if CLt:
                        nc.gpsimd.dma_gather(
                            out_ap=vg3[:, 0:CLt, :],
                            in_ap=G1[0:HALF, :],
                            idxs_ap=ilosb[:, olo[t]:olo[t] + CLt * 8],
                            num_idxs=CLt * P, num_idxs_reg=CLt * P,
                            elem_size=G1W, single_packet=False)
                    # alpha_dst values of this tile's dsts, per partition
                    adt = pmed.tile([P, H], F16, tag="adt")
                    nc.gpsimd.indirect_dma_start(
                        out=adt[:], out_offset=None, in_=G1,
                        in_offset=IndirectOffsetOnAxis(ap=adr[:, t:t + 1],
                                                       axis=0),
                        element_offset=C1 + H)
                    msk = pmsk.tile([P, SMAX * P], F16, tag="msk")
                    nc.sync.dma_start(
                        out=msk[:, 0:St * P],
                        in_=ovmask[:, omo[t]:omo[t] + St * P])
                    mskT = pmsk.tile([P, SMAX * P], F16, tag="mskT")
                    nc.scalar.dma_start(
                        out=mskT[:, 0:St * P],
                        in_=ovmaskT[:, omo[t]:omo[t] + St * P])

                    # per-slot alpha_dst for every chunk: maskT @ adt
                    adp = pbpa.tile([P, SMAX * H], F32, space="PSUM",
                                    tag="adp")
                    for k in range(St):
                        nc.tensor.matmul(
                            adp[:, k * H:(k + 1) * H],
                            lhsT=mskT[:, k * P:(k + 1) * P],
                            rhs=adt[:], start=True, stop=True)
                    alp = pmed.tile([P, SMAX * H], F16, tag="alp")
                    alp3 = alp[:].rearrange("p (k h) -> p k h", h=H)
                    nc.vector.tensor_tensor(
                        out=alp3[:, 0:St, :],
                        in0=vg3[:, 0:St, C1:C1 + H],
                        in1=adp[:].rearrange("p (k h) -> p k h", h=H)
                            [:, 0:St, :],
                        op=OP.add)
                    # e = exp(lrelu(alpha)) * vmask
                    asc = pmed.tile([P, SMAX * H], F16, tag="asc")
                    nc.vector.tensor_scalar(out=asc[:, 0:St * H],
                                            in0=alp[:, 0:St * H],
                                            scalar1=cfg.NEG, scalar2=None,
                                            op0=OP.mult)
                    lrl = pmed.tile([P, SMAX * H], F16, tag="lrl")
                    nc.vector.tensor_tensor(out=lrl[:, 0:St * H],
                                            in0=alp[:, 0:St * H],
                                            in1=asc[:, 0:St * H], op=OP.max)
                    ee = pmed.tile([P, SMAX * H], F32, tag="ee")
                    nc.scalar.activation(out=ee[:, 0:St * H],
                                         in_=lrl[:, 0:St * H], func=AF.Exp)
                    eeh = pmed.tile([P, SMAX * H], F16, tag="eeh")
                    nc.vector.tensor_tensor(
                        out=eeh[:, 0:St * H].rearrange("p (k h) -> p k h",
                                                       h=H),
                        in0=ee[:, 0:St * H].rearrange("p (k h) -> p k h",
                                                      h=H),
                        in1=vmsb[:, ovo[t]:ovo[t] + St]
                            .rearrange("p (k o) -> p k o", o=1)
                            .to_broadcast([P, St, H]),
                        op=OP.mult)
                    eeh3 = eeh[:].rearrange("p (k h) -> p k h", h=H)
                    # rhs = [e*h | e]
                    xx = pxx.tile([P, SMAX * (C1 + H)], F16, tag="xx")
                    xx3 = xx[:].rearrange("p (k c) -> p k c", c=C1 + H)
                    nc.vector.tensor_copy(out=xx3[:, 0:St, C1:C1 + H],
                                          in_=eeh3[:, 0:St, :])
                    nc.vector.tensor_tensor(
                        out=xx3[:, 0:St, 0:C1].rearrange(
                            "p k (h c) -> p k h c", c=HID),
                        in0=vg3[:, 0:St, 0:C1].rearrange(
                            "p k (h c) -> p k h c", c=HID),
                        in1=eeh[:, 0:St * H].rearrange(
                            "p (k h o) -> p k h o", h=H, o=1)
                            .to_broadcast([P, St, H, HID]),
                        op=OP.mult)
                    ps = pbp.tile([P, C1 + H], F32, space="PSUM", tag="ps")
                    for k in range(St):
                        nc.tensor.matmul(ps[:], lhsT=msk[:, k * P:(k + 1) * P],
                                         rhs=xx3[:, k, :],
                                         start=(k == 0), stop=(k == St - 1))
                    rec = pmed.tile([P, H], F32, tag="rec")
                    nc.vector.reciprocal(out=rec[:ndst, :],
                                         in_=ps[:ndst, C1:C1 + H])
                    o1 = pmed.tile([P, C1], F16, tag="o1")
                    if ndst < P:
                        nc.vector.memset(o1[:], 0.0)
                    for h in range(H):
                        nc.vector.tensor_scalar(
                            out=o1[:ndst, h * HID:(h + 1) * HID],
                            in0=ps[:ndst, h * HID:(h + 1) * HID],
                            scalar1=rec[:ndst, h:h + 1], scalar2=0.0,
                            op0=OP.mult, op1=OP.max)
                    # layer-2 row build: transpose + project
                    tp = pcpt.tile([P, P], F16, space="PSUM", tag="tp")
                    nc.tensor.transpose(out=tp[:], in_=o1[:],
                                        identity=idsb[:])
                    o1t = pmed.tile([P, P], F16, tag="o1t")
                    nc.vector.tensor_copy(out=o1t[:], in_=tp[:])
                    hp = pcp.tile([P, G2C], F32, space="PSUM", tag="hp")
                    nc.tensor.matmul(hp[:], lhsT=o1t[:], rhs=w2ext[:],
                                     start=True, stop=True)
                    g2h = pmed.tile([P, G2C], F16, tag="g2h")
                    nc.vector.tensor_tensor(out=g2h[:ndst, :],
                                            in0=hp[:ndst, :],
                                            in1=b2rep[:ndst, :], op=OP.add)
                    nc.vector.tensor_copy(out=adt2sb[:ndst, t:t + 1],
                                          in_=g2h[:ndst, C2 + 1:C2 + 2])
                    nc.sync.dma_start(out=G2c[t * P:t * P + ndst, :],
                                      in_=g2h[:ndst, :])
                    if t + 1 in TSPLIT:
                        # partial table replication overlaps the rest of B
                        j = TSPLIT.index(t + 1)
                        nc.gpsimd.collective_compute(
                            "AllGather", OP.bypass,
                            replica_groups=[list(range(cfg.NC))],
                            ins=[G2c[ROW0[j]:ROW0[j + 1], :]],
                            outs=[G2cfs[j]])
                        _relayout(j)

            nc.gpsimd.collective_compute(
                "AllGather", OP.bypass,
                replica_groups=[list(range(cfg.NC))],
                ins=[G2c[ROW0[NCH - 1]:ROW0[NCH], :]], outs=[G2cfs[NCH - 1]])
            _relayout(NCH - 1)

            # ---- phase D: layer-2 edge aggregation (1 head) ---------------
            with (
                tc.tile_pool(name="pdp", bufs=2, space="PSUM") as pdp,
                tc.tile_pool(name="pdpa", bufs=2, space="PSUM") as pdpa,
            ):
                for t in range(NT):
                    ndst = P if t < NT - 1 else cfg.LAST
                    CLt, CHt, St = CLs[t], CHs[t], Ss[t]
                    vg = pdig.tile([P, SMAX * G2W], F16, tag="vg2")
                    vg3 = vg[:].rearrange("p (k c) -> p k c", c=G2W)
                    if CLt:
                        nc.gpsimd.dma_gather(
                            out_ap=vg3[:, 0:CLt, :],
                            in_ap=G2[0:HALF, :],
                            idxs_ap=ilosb[:, olo[t]:olo[t] + CLt * 8],
                            num_idxs=CLt * P, num_idxs_reg=CLt * P,
                            elem_size=G2W, single_packet=False)
                    if CHt:
                        nc.gpsimd.dma_gather(
                            out_ap=vg3[:, CLt:St, :], in_ap=G2[HALF:NROWS, :],
                            idxs_ap=ihisb[:, ohi[t]:ohi[t] + CHt * 8],
                            num_idxs=CHt * P, num_idxs_reg=CHt * P,
                            elem_size=G2W, single_packet=False)
                    adt = adt2sb[:, t:t + 1]
                    msk = pdmk.tile([P, SMAX * P], F16, tag="msk2")
                    nc.sync.dma_start(
                        out=msk[:, 0:St * P],
                        in_=ovmask[:, omo[t]:omo[t] + St * P])
                    mskT = pdmk.tile([P, SMAX * P], F16, tag="mskT2")
                    nc.scalar.dma_start(
                        out=mskT[:, 0:St * P],
                        in_=ovmaskT[:, omo[t]:omo[t] + St * P])

                    adp = pdpa.tile([P, SMAX], F32, space="PSUM", tag="adp2")
                    for k in range(St):
                        nc.tensor.matmul(
                            adp[:, k:k + 1],
                            lhsT=mskT[:, k * P:(k + 1) * P],
                            rhs=adt, start=True, stop=True)
                    alp = pdmd.tile([P, SMAX], F16, tag="alp2")
                    alp3 = alp[:].rearrange("p (k o) -> p k o", o=1)
                    nc.vector.tensor_tensor(
                        out=alp3[:, 0:St, :],
                        in0=vg3[:, 0:St, C2:C2 + 1],
                        in1=adp[:].rearrange("p (k o) -> p k o", o=1)
                            [:, 0:St, :],
                        op=OP.add)
                    asc = pdmd.tile([P, SMAX], F16, tag="asc2")
                    nc.vector.tensor_scalar(out=asc[:, 0:St],
                                            in0=alp[:, 0:St],
                                            scalar1=cfg.NEG, scalar2=None,
                                            op0=OP.mult)
                    lrl = pdmd.tile([P, SMAX], F16, tag="lrl2")
                    nc.vector.tensor_tensor(out=lrl[:, 0:St],
                                            in0=alp[:, 0:St],
                                            in1=asc[:, 0:St], op=OP.max)
                    ee = pdmd.tile([P, SMAX], F32, tag="ee2")
                    nc.scalar.activation(out=ee[:, 0:St], in_=lrl[:, 0:St],
                                         func=AF.Exp)
                    eeh = pdmd.tile([P, SMAX], F16, tag="eeh2")
                    nc.vector.tensor_tensor(out=eeh[:, 0:St],
                                            in0=ee[:, 0:St],
                                            in1=vmsb[:, ovo[t]:ovo[t] + St],
                                            op=OP.mult)
                    xx = pdxx.tile([P, SMAX * (C2 + 1)], F16, tag="xx2")
                    xx3 = xx[:].rearrange("p (k c) -> p k c", c=C2 + 1)
                    nc.vector.tensor_copy(
                        out=xx3[:, 0:St, C2:C2 + 1],
                        in_=eeh[:, 0:St].rearrange("p (k o) -> p k o", o=1))
                    nc.vector.tensor_tensor(
                        out=xx3[:, 0:St, 0:C2],
                        in0=vg3[:, 0:St, 0:C2],
                        in1=eeh[:, 0:St].rearrange("p (k o) -> p k o", o=1)
                            .to_broadcast([P, St, C2]),
                        op=OP.mult)
                    ps = pdp.tile([P, C2 + 1], F32, space="PSUM", tag="ps2")
                    for k in range(St):
                        nc.tensor.matmul(ps[:], lhsT=msk[:, k * P:(k + 1) * P],
                                         rhs=xx3[:, k, :],
                                         start=(k == 0), stop=(k == St - 1))
                    rec = pdmd.tile([P, 1], F32, tag="rec2")
                    nc.vector.reciprocal(out=rec[:ndst, :],
                                         in_=ps[:ndst, C2:C2 + 1])
                    o2 = pdmd.tile([P, C2], F32, tag="o2")
                    nc.vector.tensor_scalar(out=o2[:ndst, :],
                                            in0=ps[:ndst, 0:C2],
                                            scalar1=rec[:ndst, :],
                                            scalar2=None, op0=OP.mult)
                    og = pdmd.tile([P, C2], F32, tag="og")
                    nc.scalar.activation(out=og[:ndst, :], in_=o2[:ndst, :],
                                         func=AF.Sigmoid)
                    nc.sync.dma_start(out=out[t * P:t * P + ndst, :],
                                      in_=og[:ndst, :])

            _stack.close()

    nc.compile()
    return nc


_CACHE: dict = {}


def _get_module(cfg, plan):
    key = (cfg.N, cfg.E, plan.CLs, plan.CHs)
    if key not in _CACHE:
        _CACHE[key] = _build(cfg, plan)
    return _CACHE[key]


def _run(cfg, inputs, trace=False):
    plan, common, per_core = _prep_host(
        cfg, inputs["x"], inputs["edge_index"], inputs["W1"],
        inputs["a_src1"], inputs["a_dst1"], inputs["b1"], inputs["W2"],
        inputs["a_src2"], inputs["a_dst2"], inputs["b2"])
    nc = _get_module(cfg, plan)
    in_maps = [dict(common, **pc) for pc in per_core]
    res = run_bass_kernel_spmd(nc, in_maps, core_ids=list(range(cfg.NC)),
                               trace=trace)
    shards = [np.asarray(res.results[c]["out"])[:cfg.NPC]
              for c in range(cfg.NC)]
    full = np.concatenate(shards, axis=0).astype(np.float32)
    return (full, res) if trace else full


def kernel(**inputs) -> np.ndarray:
    cfg = GATConfig(n=50000, e=800000)
    return _run(cfg, inputs)
